# revision 45
# baseline (speedup 1.0000x reference)
"""Trainium2 Bass kernel for nn_DFlashSelfAttention (block-sparse GQA attention).

Self-contained: builds the Bass module once, shards inputs over 8 NeuronCores
(sequence-parallel), runs via run_bass_kernel_spmd, reassembles full output.
"""

import sys as _sys
for _p in ("/opt/trn_rl_repo",):
    if _p not in _sys.path:
        _sys.path.insert(0, _p)

"""Bass/Tile kernel for DFlashSelfAttention (block-diagonal causal attention).

Sharding: sequence-parallel over L (2048 -> 8 cores x 256 positions).
Attention is block-diagonal with BLOCK=16, so positions never interact
across 16-blocks; a 256-position slice (16 blocks) is fully independent.

The two big GEMMs (X@Wqkv and A@Wo) run in fp8(e4m3) DoubleRow perf mode
(2 contraction rows/cycle) with residual compensation: each operand O is
split host-side (or on-device for A) into Oh = e4m3(s*O) and
Ol = e4m3(s*O - Oh) at the SAME scale, so the three product terms
Oh*Wh + Ol*Wh + Oh*Wl accumulate directly in PSUM (lo*lo dropped).
This costs 3 DoubleRow passes = 0.75x the fp16 matmul cycles at ~1.8e-3
final relative error (fp16 everywhere gives 5.4e-4; tolerance is 2e-2).

Per-core pipeline (T = 512 rows = 2 batches x 256 positions):
  phase A: Q = X @ Wq, 3-pass fp8 over all 8 psum banks.
  phase B: KV = X @ Wkv into banks freed by Q->SBUF drains (descale 2^-17
    folded into the drain copies; V additionally scaled by 32 = SA).
  per 128-token chunk: fp16 RMS-norm+RoPE (rstd = exp(-ln(v)/2) keeps the
    ACT engine on one table set), PE pair-transposes of Q^T/K^T, then
    GROUP-BATCHED attention: per kv-head group g of 4 query heads one
    [128,512] psum tile holds mask+S for (head, qtok); one ACT exp; column
    sums via GPSIMD partition_all_reduce; one AV matmul lands at psum
    partitions (g%2)*64; normalization = per-pair DVE reciprocal + one DVE
    multiply straight from PSUM into A^T (fp16, = 32*A), then ACT cast to
    fp8 hi + DVE subtract to fp8 lo.
  emit: Y^T = Wo^T @ A^T per 256-token half (3-pass fp8 DoubleRow),
    interleaved with the chunk-2/3 attention chains; fp16 DRAM [4096, 512];
    host transposes back.

Attention math stays fp16 (1 cy/row): RMS-norm weights and the sqrt(1/8)
attention scale are folded into host-precomputed rope tables.
"""

import ml_dtypes
import numpy as np

import concourse.bass as bass
import concourse.mybir as mybir
import concourse.tile as tile
from concourse import bacc
from concourse.bass_isa import ReduceOp
from concourse.masks import make_identity

F32 = mybir.dt.float32
F16 = mybir.dt.float16
F8 = mybir.dt.float8e4
E4 = ml_dtypes.float8_e4m3

P = 128
HID = 4096
KO = HID // P          # 32 k-chunks over hidden
KP = KO // 2           # 16 DoubleRow k-pairs
T = 512                # rows per core: 2 batches x 256 positions
NM = T // P            # 4 t-chunks
NH = 16
NKV = 4
HD = 64
H2 = HD // 2
QD = NH * HD           # 1024
KVD = 2 * NKV * HD     # 512 (k 256 | v 256)
EPS = 1e-6

SX = 32.0              # hidden-states fp8 scale
SW = 4096.0            # weights fp8 scale (Wqkv, Wo)
SA = 32.0              # attention-out fp8 scale
DS = 1.0 / (SX * SW)   # qkv psum descale = 2^-17 (also = 1/(SA*SW) for emit)
DSV = DS * SA          # v drain: descale then re-scale by SA

DR = mybir.MatmulPerfMode.DoubleRow

# Q-head permutation: position p holds original head PERM[p]. Even positions
# carry heads of even kv parity groups; transposed pair-tiles then expose each
# kv group's 4 heads as one contiguous [64, 4, 128] moving operand.
PERM = [0, 4, 1, 5, 2, 6, 3, 7, 8, 12, 9, 13, 10, 14, 11, 15]


def _pin_act_tables():
    """Make every activation resolve to the natural_log_exp_and_others set so
    the act-table pass emits one load instead of ping-ponging between the
    exp-only and ln-only tables. Indices (act_func_set_id) are preserved."""
    import concourse.bacc as _bacc_mod
    from concourse import hw_specs as _hw
    real = _hw.get_activation_tables

    def pinned(arch):
        t = real(arch)
        keep = "natural_log_exp_and_others"
        if keep not in t:
            return t
        return {nm: (fns if nm == keep else set()) for nm, fns in t.items()}

    _bacc_mod.get_activation_tables = pinned
    return _bacc_mod, real


def build_nc(name="dfa"):
    _bacc_mod, _real_gat = _pin_act_tables()
    try:
        return _build_nc_inner(name)
    finally:
        _bacc_mod.get_activation_tables = _real_gat


def _build_nc_inner(name="dfa"):
    nc = bacc.Bacc(None, target_bir_lowering=False, name=name)

    # hi/lo fp8 pairs packed on dim 1 (v: 0=hi, 1=lo)
    xt = nc.dram_tensor("xt", [P, 2, KO, T], F8, kind="ExternalInput")
    wq = nc.dram_tensor("wq", [P, 2, KO, QD], F8, kind="ExternalInput")
    wkv = nc.dram_tensor("wkv", [P, 2, KO, KVD], F8, kind="ExternalInput")
    wo = nc.dram_tensor("wo", [P, 2, 32, 8, P], F8, kind="ExternalInput")
    tabs = nc.dram_tensor("tabs", [T, 4 * HD], F16, kind="ExternalInput")
    mask4 = nc.dram_tensor("mask4", [P, 2 * 4 * P], F8, kind="ExternalInput")
    identw = nc.dram_tensor("identw", [P, 2 * P], F8, kind="ExternalInput")
    yt = nc.dram_tensor("yt", [HID, T], F16, kind="ExternalOutput")

    tabs_r = tabs.rearrange("(m p) d -> p m d", p=P)
    yt_r = yt.rearrange("(mo p) t -> p mo t", p=P)

    from contextlib import ExitStack
    with tile.TileContext(nc) as tc, ExitStack() as ctx:
        consts = ctx.enter_context(tc.tile_pool(name="consts", bufs=1))
        xt_pool = ctx.enter_context(tc.tile_pool(name="xt", bufs=1))
        wstream = ctx.enter_context(tc.tile_pool(name="wstream", bufs=4))
        acts = ctx.enter_context(tc.tile_pool(name="acts", bufs=1))
        rope_tmp = ctx.enter_context(tc.tile_pool(name="rope_tmp", bufs=1))
        attn_tmp = ctx.enter_context(tc.tile_pool(name="attn_tmp", bufs=2))
        ystage = ctx.enter_context(tc.tile_pool(name="ystage", bufs=2))
        pp = ctx.enter_context(tc.tile_pool(name="pp", bufs=1, space="PSUM"))

        def ptile(shape, bank, name, dtype=F32):
            tot = 512 if dtype == F32 else 1024
            pad = list(shape)
            pad[-1] = max(1, tot // int(np.prod(shape[1:-1])))
            return pp.tile(shape, dtype, tag=f"b{bank}", name=name,
                           padded_shape=pad)

        # ---- leading DMAs: hi parts first so the hi*hi pass of k-pair 0 can
        # start after half the bytes; few large pieces — the serialized
        # ~625ns HWDGE config per DMA, not transfer time, is what delays
        # phase A ----
        # constants are engine-local — issuing them first costs the DMA
        # stream nothing and un-gates the warmup matmuls
        ident = consts.tile([P, P], F16)
        make_identity(nc, ident)
        eps_t = consts.tile([P, 1], F32)
        nc.vector.memset(eps_t, EPS)

        xt_sb = xt_pool.tile([P, 2, KO, T], F8)
        wq_b0 = wstream.tile([P, 2, 4, QD], F8, tag="wq4", bufs=3,
                             name="wq_b0")
        nc.sync.dma_start(xt_sb[:, 0:1, 0:2, :], xt[:, 0:1, 0:2, :])
        nc.sync.dma_start(wq_b0[:, 0:1, 0:2, :], wq[:, 0:1, 0:2, :])
        nc.sync.dma_start(xt_sb[:, 1:2, 0:2, :], xt[:, 1:2, 0:2, :])
        nc.sync.dma_start(wq_b0[:, 1:2, 0:2, :], wq[:, 1:2, 0:2, :])
        nc.sync.dma_start(wq_b0[:, :, 2:4, :], wq[:, :, 2:4, :])
        nc.sync.dma_start(xt_sb[:, :, 2:4, :], xt[:, :, 2:4, :])

        # wq coarse batches (bufs=4 so FIFO WAR stalls never starve phase A).
        # wkv/mask/tabs are deferred past the phase-A window: its DMA demand
        # (wq+xt+wkv) would exceed supply by ~1.1us per batch; wkv tile kb is
        # not read until b_phase, which trails phase A by >=4us
        wq_coarse = []
        wkv_tiles = []
        mask_sb = consts.tile([P, 2, 4, P], F8)
        identw_sb = consts.tile([P, 2, P], F8)
        tabs_sb = consts.tile([P, NM, 4, HD], F16)

        def wkv_load(kb):
            wkvt = wstream.tile([P, 2, 4, KVD], F8, tag="wkv", bufs=8,
                                name=f"wkv_k{kb}")
            nc.sync.dma_start(wkvt[:], wkv[:, :, kb * 4:(kb + 1) * 4, :])
            wkv_tiles.append(wkvt)

        for kb in range(1, 8):
            wqt = wstream.tile([P, 2, 4, QD], F8, tag="wq4", bufs=3,
                               name=f"wq_b{kb}")
            nc.sync.dma_start(wqt[:], wq[:, :, kb * 4:(kb + 1) * 4, :])
            wq_coarse.append(wqt)
            nc.sync.dma_start(xt_sb[:, :, kb * 4:(kb + 1) * 4, :],
                              xt[:, :, kb * 4:(kb + 1) * 4, :])
            if kb >= 3:
                wkv_load(kb - 3)
        # tabs/mask feed the rope/scores chains that now start right at the
        # head of phase B — ahead of the tail wkv tiles
        nc.sync.dma_start(
            tabs_sb[:], tabs_r.rearrange("p m (f d) -> p m f d", d=HD))
        nc.sync.dma_start(
            mask_sb[:], mask4.rearrange("p (v f q) -> p v f q", v=2, q=P))
        nc.sync.dma_start(
            identw_sb[:], identw.rearrange("p (v q) -> p v q", v=2))
        for kb in range(5, 8):
            wkv_load(kb)

        # PE warmup: anchor the p-state ramp while the first DMAs land
        warm_src = consts.tile([P, 512], F16)
        nc.vector.memset(warm_src, 0.0)
        for w in range(3):
            wps = ptile([P, 512], 0, f"warm{w}")
            nc.tensor.matmul(wps[:], warm_src[:, 0:P], warm_src[:],
                             start=True, stop=True)

        # ---- phase A: Q = X @ Wq, 3-pass fp8 DoubleRow over all 8 banks ----
        ps_a = [[ptile([P, 512], 2 * m + s, f"psa{m}_{s}")
                 for s in range(2)] for m in range(NM)]

        PASSES = ((0, 0), (1, 0), (0, 1))   # (vx, vw): hi*hi, lo*hi, hi*lo

        def a_matmuls(kp, wq_tile, jo, passes=(0, 1, 2)):
            """kp: global k-pair (0..15); wq_tile sliced at pair offset jo."""
            for m in range(NM):
                ms = slice(m * P, (m + 1) * P)
                for s in range(2):
                    ss = slice(s * 512, (s + 1) * 512)
                    for pi in passes:
                        vx, vw = PASSES[pi]
                        nc.tensor.matmul(
                            ps_a[m][s][:],
                            xt_sb[:, vx, 2 * kp:2 * kp + 2, ms],
                            wq_tile[:, vw, 2 * jo:2 * jo + 2, ss],
                            start=(kp == 0 and pi == 0),
                            stop=(kp == KP - 1 and pi == 2), perf_mode=DR)

        a_matmuls(0, wq_b0, 0, passes=(0,))    # needs only hi DMAs
        a_matmuls(0, wq_b0, 0, passes=(1,))    # + xt lo
        a_matmuls(0, wq_b0, 0, passes=(2,))    # + wq lo
        a_matmuls(1, wq_b0, 1)
        for kb in range(1, 8):
            for jo in range(2):
                a_matmuls(2 * kb + jo, wq_coarse[kb - 1], jo)

        # ---- Q drains (DVE/ACT split, descale 2^-17) + phase B + KV drains
        qn = [acts.tile([P, NH, HD], F16, tag=f"qn{m}", name=f"qn{m}")
              for m in range(NM)]

        def qn_copy(m):
            nc.vector.tensor_scalar_mul(
                qn[m][:, 0:8, :],
                ps_a[m][0][:].rearrange("p (h d) -> p h d", d=HD), DS)
            nc.scalar.activation(
                qn[m][:, 8:16, :],
                ps_a[m][1][:].rearrange("p (h d) -> p h d", d=HD),
                mybir.ActivationFunctionType.Copy, scale=DS)

        ps_b = []
        kn = []
        v_sb = []

        def b_phase(m):
            pb = ptile([P, KVD], 6 + m % 2, f"psb{m}")
            ps_b.append(pb)
            ms = slice(m * P, (m + 1) * P)
            ni = 0
            for kb in range(8):
                for jo in range(2):
                    kp = 2 * kb + jo
                    for pi, (vx, vw) in enumerate(((0, 0), (1, 0), (0, 1))):
                        nc.tensor.matmul(
                            pb[:], xt_sb[:, vx, 2 * kp:2 * kp + 2, ms],
                            wkv_tiles[kb][:, vw, 2 * jo:2 * jo + 2, :],
                            start=(ni == 0), stop=(ni == 3 * KP - 1),
                            perf_mode=DR)
                        ni += 1

        def kv_drain(m):
            knt = acts.tile([P, NKV, HD], F16, tag=f"kn{m}", name=f"kn{m}")
            nc.vector.tensor_scalar_mul(
                knt[:], ps_b[m][:, 0:256].rearrange("p (h d) -> p h d", d=HD),
                DS)
            kn.append(knt)
            vt = acts.tile([P, 256], F16, tag=f"v{m}", name=f"v{m}")
            nc.scalar.activation(vt[:], ps_b[m][:, 256:512],
                                 mybir.ActivationFunctionType.Copy, scale=DSV)
            v_sb.append(vt)

        qn_copy(3)      # frees banks 6/7 for the b_phase rotation first
        qn_copy(0)
        b_phase(0)
        kv_drain(0)
        qn_copy(1)
        qn_copy(2)

        # ---- rope helpers (fp16 math; rstd via ln/exp on ACT) ----
        def stats(src, nh, m, tag):
            sq = rope_tmp.tile([P, nh, HD], F16, tag=f"sq{nh}", bufs=1,
                               name=f"sq_{tag}")
            nc.vector.tensor_mul(sq[:], src[:], src[:])
            ssq = rope_tmp.tile([P, nh], F32, tag=f"ssq{nh}", bufs=2,
                                name=f"ssq_{tag}")
            nc.vector.reduce_sum(ssq[:], sq[:], axis=mybir.AxisListType.X)
            lnv = rope_tmp.tile([P, nh], F32, tag=f"lnv{nh}", bufs=2,
                                name=f"lnv_{tag}")
            nc.scalar.activation(lnv[:], ssq[:],
                                 mybir.ActivationFunctionType.Ln,
                                 bias=eps_t[:], scale=1.0 / HD)
            rstd = rope_tmp.tile([P, nh], F16, tag=f"rstd{nh}", bufs=2,
                                 name=f"rstd_{tag}")
            nc.scalar.activation(rstd[:], lnv[:],
                                 mybir.ActivationFunctionType.Exp, scale=-0.5)
            return rstd

        def rope_apply(src, rstd, m, nh, cf, sf, tag):
            qn2 = rope_tmp.tile([P, nh, HD], F16, tag=f"qn2_{nh}", bufs=2,
                                name=f"qn2_{tag}")
            nc.vector.tensor_mul(qn2[:], src[:],
                                 rstd[:, :, None].to_broadcast((P, nh, HD)))
            ctab = tabs_sb[:, m, cf, :]
            stab = tabs_sb[:, m, sf, :]
            o1 = rope_tmp.tile([P, nh, HD], F16, tag=f"o1_{nh}", bufs=1,
                               name=f"o1_{tag}")
            nc.vector.tensor_mul(o1[:], qn2[:],
                                 ctab[:, None, :].to_broadcast((P, nh, HD)))
            o2 = rope_tmp.tile([P, nh, HD], F16, tag=f"o2_{nh}", bufs=1,
                               name=f"o2_{tag}")
            nc.vector.tensor_mul(
                o2[:, :, 0:H2], qn2[:, :, H2:HD],
                stab[:, None, 0:H2].to_broadcast((P, nh, H2)))
            nc.vector.tensor_mul(
                o2[:, :, H2:HD], qn2[:, :, 0:H2],
                stab[:, None, H2:HD].to_broadcast((P, nh, H2)))
            out = rope_tmp.tile([P, nh * HD], F16, tag=f"ro_{nh}", bufs=2,
                                name=f"ro_{tag}")
            nc.vector.tensor_add(out[:], o1[:].rearrange("p h d -> p (h d)"),
                                 o2[:].rearrange("p h d -> p (h d)"))
            return out

        krT = {}
        qrT = {}
        rope_out_k = {}
        rope_out_q = {}

        def rope_k(m):
            """DVE/ACT-only: rms-norm + rope for k of chunk m (needs kv m)."""
            rstd_k = stats(kn[m], NKV, m, f"k{m}")
            rope_out_k[m] = rope_apply(kn[m], rstd_k, m, NKV, 2, 3, f"k{m}")

        def rope_q(m):
            """Needs only qn (phase-A drains) + tabs — hoistable early."""
            rstd_q = stats(qn[m], NH, m, f"q{m}")
            rope_out_q[m] = rope_apply(qn[m], rstd_q, m, NH, 0, 1, f"q{m}")

        def rope_kq(m):
            rope_k(m)
            rope_q(m)

        def trans_kq(m):
            """PE transposes + ACT psum drains for chunk m (needs
            rope_kq(m)); XBAR DMA transposes were tried and regress — their
            transfers queue behind 1.5us bulk weight DMAs on the shared DMA
            engines right when the S matmuls need them."""
            kr = rope_out_k[m]
            qr = rope_out_q[m]
            kps = ptile([P, 2, P], 4, f"krT_ps{m}", F16)
            for j in range(2):
                nc.tensor.matmul(kps[:, j, :], kr[:, j * P:(j + 1) * P],
                                 ident[:], is_transpose=True)
            kt = acts.tile([P, 2, P], F16, tag=f"krT{m}", name=f"krT{m}")
            nc.scalar.copy(kt[:], kps[:])
            krT[m] = kt
            qps = ptile([P, 8, P], 5, f"qrT_ps{m}", F16)
            for j in range(8):
                nc.tensor.matmul(qps[:, j, :],
                                 qr[:, 2 * j * HD:(2 * j + 2) * HD],
                                 ident[:], is_transpose=True)
            qt = acts.tile([P, 8, P], F16, tag=f"qrT{m}", name=f"qrT{m}")
            nc.scalar.copy(qt[:], qps[:])
            qrT[m] = qt

        # A^T accumulator (= SA * A): partition (g%2)*64+d, col c=(g//2)*4+i,
        # token t. at16 fp16 + on-device fp8 hi/lo split for the emit GEMM.
        at16 = acts.tile([P, 8, T], F16, tag="at16")
        at8h = acts.tile([P, 8, T], F8, tag="at8h")
        at8l = acts.tile([P, 8, T], F8, tag="at8l")

        # ---- group-batched attention, split into a scores half and an AV
        # half so other PE work (phase B tail, transposes, emits) can sit
        # between them and hide the exp->gpsimd chain latency ----
        attn_state = {}

        def attn_scores(m):
            sums = attn_tmp.tile([P, 4, 512], F32, tag="sums", bufs=1,
                                 name=f"sums{m}")
            # pair layout: partition half (g%2)*64 of column gp holds group
            # g's reciprocal, matching o_ps[gp]'s layout so one [128,512]
            # multiply normalizes both groups of a pair at once
            rcp = attn_tmp.tile([P, 2, 512], F32, tag="rcp", bufs=1,
                                name=f"rcp{m}")
            ests = []
            attn_state[m] = (rcp, {}, sums, ests)
            for g in range(4):
                s_ps = ptile([P, 512], g % 2, f"s{m}_{g}")
                nc.tensor.matmul(s_ps[:], identw_sb[:],
                                 mask_sb[:].rearrange("p v f q -> p v (f q)"),
                                 start=True, stop=False, perf_mode=DR)
                base = (g % 2) * HD
                gp = g // 2
                nc.tensor.matmul(s_ps[:],
                                 krT[m][base:base + HD, gp, :],
                                 qrT[m][base:base + HD, 4 * gp:4 * gp + 4, :],
                                 start=False, stop=True)
                est = attn_tmp.tile([P, 4, P], F16, tag="est", bufs=4,
                                    name=f"est{m}_{g}")
                nc.scalar.activation(est[:], s_ps[:],
                                     mybir.ActivationFunctionType.Exp)
                nc.gpsimd.partition_all_reduce(sums[:, g, :], est[:],
                                               channels=P,
                                               reduce_op=ReduceOp.add)
                ests.append(est)

        def attn_avs(m):
            rcp, o_ps, sums, ests = attn_state[m]
            for g in range(4):
                base = (g % 2) * HD
                gp = g // 2
                if gp not in o_ps:
                    o_ps[gp] = ptile([P, 512], 2 + gp, f"o{m}_{gp}")
                nc.tensor.matmul(o_ps[gp][base:base + HD, :],
                                 v_sb[m][:, g * HD:(g + 1) * HD], ests[g][:],
                                 start=True, stop=True)
                if g % 2 == 1:
                    # per-pair reciprocals into the pair layout; lane-locked
                    # halves keep the DVE queue moving
                    nc.vector.reciprocal(rcp[0:HD, gp, :],
                                         sums[0:HD, g - 1, :])
                    nc.vector.reciprocal(rcp[HD:P, gp, :],
                                         sums[HD:P, g, :])

        def attn_core(m):
            attn_scores(m)
            attn_avs(m)

        def attn_norm(m):
            rcp, o_ps, _, _ = attn_state[m]
            ms = slice(m * P, (m + 1) * P)
            for gp in range(2):
                # one multiply normalizes both groups of the pair: partition
                # halves of o_ps/rcp line up with at16's (g%2) layout
                nc.vector.tensor_mul(
                    at16[:, 4 * gp:4 * gp + 4, ms],
                    o_ps[gp][:].rearrange("p (i t) -> p i t", t=P),
                    rcp[:, gp, :].rearrange("p (i t) -> p i t", t=P))
            # fp8 hi/lo split for the emit GEMM: cast on DVE, residual on
            # GPSIMD — keeps the ACT queue pure exp (its latency releases the
            # S psum banks) and the softmax-critical Pool sums unobstructed
            nc.vector.tensor_copy(at8h[:, :, ms], at16[:, :, ms])
            nc.gpsimd.tensor_sub(at8l[:, 0:4, ms], at16[:, 0:4, ms],
                                 at8h[:, 0:4, ms])
            nc.vector.tensor_sub(at8l[:, 4:8, ms], at16[:, 4:8, ms],
                                 at8h[:, 4:8, ms])

        # ---- emit: Y^T = Wo^T @ A^T (3-pass fp8 DoubleRow); two mo's pair
        # up per psum bank so one [P,512] copy drains them and the WAR
        # pipeline is 4 mo's deep ----
        def emit_mb(half, mb, bank, split_tail=False, wo_tile=None):
            c0 = half * 256
            tsl = slice(c0, c0 + 256)
            if wo_tile is not None:
                wo_m = wo_tile
            else:
                # mb 0/1 tiles stay resident ("wo01") for the final half-1
                # emits so the tail has no wo DMAs in front of its yt writes
                tag, bufs = ("wo01", 1) if mb < 1 else ("wo", 3)
                wo_m = wstream.tile([P, 2, 4, 8, P], F8, tag=tag, bufs=bufs,
                                    name=f"wo_m{half}_{mb}")
                nc.sync.dma_start(wo_m[:], wo[:, :, mb * 4:(mb + 1) * 4, :, :])
            ys = ystage.tile([P, 4, 256], F16, tag="ys", name="ys")
            for pair in range(2):
                ps = ptile([P, 2, 256], bank[pair], f"ps_y{half}_{mb}_{pair}")
                for sub2 in range(2):
                    sub = pair * 2 + sub2
                    ni = 0
                    for u in range(4):
                        for (va, vw) in ((0, 0), (1, 0), (0, 1)):
                            at_op = at8h if va == 0 else at8l
                            nc.tensor.matmul(
                                ps[:, sub2, :],
                                wo_m[:, vw, sub, 2 * u:2 * u + 2, :],
                                at_op[:, 2 * u:2 * u + 2, tsl],
                                start=(ni == 0), stop=(ni == 11),
                                perf_mode=DR)
                            ni += 1
                # alternate copy engines by (mb+pair) parity so consecutive
                # same-bank drains never queue behind each other
                on_dve = (mb + pair) % 2 == 0
                ys_dst = ys[:, 2 * pair:2 * pair + 2, :] \
                    .rearrange("p i t -> p (i t)")
                ps_src = ps[:].rearrange("p i t -> p (i t)")
                if on_dve:
                    nc.vector.tensor_scalar_mul(ys_dst, ps_src, DS)
                else:
                    nc.scalar.activation(ys_dst, ps_src,
                                         mybir.ActivationFunctionType.Copy,
                                         scale=DS)
                if split_tail:
                    nc.sync.dma_start(
                        yt_r[:, mb * 4 + 2 * pair:mb * 4 + 2 * pair + 2,
                             c0:c0 + 256],
                        ys[:, 2 * pair:2 * pair + 2, :])
            if not split_tail:
                nc.sync.dma_start(yt_r[:, mb * 4:(mb + 1) * 4, c0:c0 + 256],
                                  ys[:])
            return wo_m

        # ---- schedule: rope chains and transposes overlap the tail of
        # phase B; attention starts the moment B's last matmul retires;
        # each chunk's normalize+fp8-split follows its core so half-0 emits
        # (tokens 0..255 = chunks 0,1) can interleave with chunk 2/3
        # attention and fill the PE gaps of the latency-bound softmax chain
        # attention chunks pipeline INTO phase B: rope chains (DVE/ACT)
        # issue one b_phase ahead of their PE transposes, scores' exp/sums
        # latency hides behind the next b_phase's matmuls, and the emit
        # stream starts right after the last AV chain
        rope_kq(0)
        b_phase(1)
        kv_drain(1)
        rope_kq(1)
        trans_kq(0)
        b_phase(2)
        kv_drain(2)
        rope_kq(2)
        trans_kq(1)
        attn_scores(0)
        b_phase(3)       # fills the exp->gpsimd latency of chunk 0
        kv_drain(3)
        attn_avs(0)
        attn_scores(1)
        trans_kq(2)      # after scores(1): its ACT copies stay behind S1's
                         # bank-releasing exps
        attn_norm(0)     # DVE: frees o_ps banks 2/3 before rope(3) queues
        rope_kq(3)
        attn_avs(1)
        attn_scores(2)
        trans_kq(3)
        attn_norm(1)
        attn_avs(2)
        attn_scores(3)
        attn_norm(2)
        wo_01 = emit_mb(0, 0, (6, 7))
        attn_avs(3)
        emit_mb(0, 1, (4, 5))
        attn_norm(3)
        # late mb stages cover both token halves from one wo load (wo would
        # otherwise be streamed twice: ~15us of DMA traffic saved); half-0
        # rotates (6,7)/(4,5) and half-1 (0,1)/(2,3) so four banks pipeline
        H1B = [(0, 1), (2, 3)]
        H0B = [(6, 7), (4, 5)]
        wo_t11 = None
        for mb in range(2, 8):
            wo_t = emit_mb(0, mb, H0B[mb % 2])
            if mb == 7:
                # prefetch the mb1 reload while the mb7 emits run so the
                # tail has no wo DMA in front of its yt writes
                wo_t11 = wstream.tile([P, 2, 4, 8, P], F8, tag="wo", bufs=3,
                                      name="wo_m1_reload")
                nc.sync.dma_start(wo_t11[:], wo[:, :, 4:8, :, :])
            emit_mb(1, mb, H1B[mb % 2], wo_tile=wo_t)
        emit_mb(1, 0, (0, 1), split_tail=True, wo_tile=wo_01)
        emit_mb(1, 1, (2, 3), split_tail=True, wo_tile=wo_t11)

    nc.finalize()
    return nc


def _split8(a, s):
    """fp8 hi/lo residual split at common scale s: a*s ~= hi + lo."""
    hi = (a * s).astype(E4)
    lo = (a * s - hi.astype(np.float32)).astype(E4)
    return hi, lo


def host_inputs(inputs, core):
    """Build the per-core DRAM input map from full problem inputs."""
    hs = np.asarray(inputs["hidden_states"], np.float32)
    am = np.asarray(inputs["attention_mask"], np.float32)
    cos = np.asarray(inputs["cos"], np.float32)
    sin = np.asarray(inputs["sin"], np.float32)
    Wqkv = np.asarray(inputs["Wqkv"], np.float32)
    Wo = np.asarray(inputs["Wo"], np.float32)
    qw = np.asarray(inputs["q_norm_w"], np.float32)
    kw = np.asarray(inputs["k_norm_w"], np.float32)

    LS = 256
    ls = slice(core * LS, (core + 1) * LS)
    X = hs[:, ls, :].reshape(T, HID)
    xt_f = np.ascontiguousarray(X.T)                      # [HID, T]
    xh, xl = _split8(xt_f, SX)
    # pack [HID, T] -> [P, 2, KO, T]
    xt8 = np.stack([xh.reshape(KO, P, T), xl.reshape(KO, P, T)], axis=0) \
        .transpose(2, 0, 1, 3)

    cos_c = cos[:, ls, :].reshape(T, HD)
    sin_c = sin[:, ls, :].reshape(T, HD)
    sq = float(HD) ** -0.25  # sqrt(1/sqrt(HD)) = sqrt(1/8)
    swap = np.concatenate([np.arange(32, 64), np.arange(0, 32)])
    sign = np.concatenate([-np.ones(32, np.float32), np.ones(32, np.float32)])

    tabs = np.empty((T, 4, HD), np.float32)
    tabs[:, 0, :] = cos_c * qw[None, :] * sq
    tabs[:, 1, :] = sin_c * qw[swap][None, :] * sign[None, :] * sq
    tabs[:, 2, :] = cos_c * kw[None, :] * sq
    tabs[:, 3, :] = sin_c * kw[swap][None, :] * sign[None, :] * sq

    # fp8 DoubleRow mask: v0 holds mask/16 (0 or -240), v1 zeros; the
    # stationary identw v0 is 16*I so the product restores -3840 (exp -> 0)
    maskT8 = np.where(am[0, 0, :P, :P].T < -1.0, -240.0, 0.0).astype(E4)
    mask8 = np.zeros((P, 2, 4, P), E4)
    mask8[:, 0, :, :] = np.broadcast_to(maskT8[:, None, :], (P, 4, P))
    identw = np.zeros((P, 2, P), E4)
    identw[:, 0, :] = (np.eye(P, dtype=np.float32) * 16.0).astype(E4)

    wq_f = np.ascontiguousarray(
        Wqkv[:, :QD].reshape(HID, NH, HD)[:, PERM, :].reshape(HID, QD))
    wqh, wql = _split8(wq_f, SW)
    wq8 = np.stack([wqh.reshape(KO, P, QD), wql.reshape(KO, P, QD)],
                   axis=0).transpose(2, 0, 1, 3)

    wkv_f = Wqkv[:, QD:]
    wkh, wkl = _split8(wkv_f, SW)
    wkv8 = np.stack([wkh.reshape(KO, P, KVD), wkl.reshape(KO, P, KVD)],
                    axis=0).transpose(2, 0, 1, 3)

    # wo[p=(par,d), mo, c, j] = Wo[h(c,par)*64+d, mo*128+j]
    woh = Wo.reshape(NH, HD, 32, P)
    wo_np = np.empty((P, 32, 8, P), np.float32)
    for par in range(2):
        for c in range(8):
            h = 8 * (c // 4) + 4 * par + (c % 4)
            wo_np[par * 64:(par + 1) * 64, :, c, :] = woh[h]
    woh8, wol8 = _split8(wo_np, SW)
    wo8 = np.stack([woh8, wol8], axis=1)                   # [P, 2, 32, 8, P]

    m = {
        "xt": np.ascontiguousarray(xt8),
        "tabs": np.ascontiguousarray(tabs.reshape(T, 4 * HD)).astype(np.float16),
        "wq": np.ascontiguousarray(wq8),
        "wkv": np.ascontiguousarray(wkv8),
        "wo": np.ascontiguousarray(wo8),
        "mask4": np.ascontiguousarray(mask8.reshape(P, 2 * 4 * P)),
        "identw": np.ascontiguousarray(identw.reshape(P, 2 * P)),
    }
    return m


def assemble_output(yts):
    """yts: list of 8 [4096, 512] fp16 arrays -> [2, 2048, 4096] f32."""
    out = np.empty((2, 2048, HID), np.float32)
    for c, yt_ in enumerate(yts):
        sl = yt_.astype(np.float32).T.reshape(2, 256, HID)
        out[:, c * 256:(c + 1) * 256, :] = sl
    return out


_NC_CACHE = {}


def _get_nc():
    if "nc" not in _NC_CACHE:
        _NC_CACHE["nc"] = build_nc()
    return _NC_CACHE["nc"]


def _run(inputs, trace=False):
    from concourse.bass_utils import run_bass_kernel_spmd
    nc = _get_nc()
    in_maps = [host_inputs(inputs, c) for c in range(8)]
    res = run_bass_kernel_spmd(nc, in_maps, core_ids=list(range(8)),
                               trace=trace)
    out = assemble_output([res.results[c]["yt"] for c in range(8)])
    return out, res


def kernel(**inputs):
    out, _ = _run(inputs, trace=False)
    if not np.isfinite(out).all():
        # transient first-execution flake seen once on device; retry
        out, _ = _run(inputs, trace=False)
    return out


def _timed_runs(inputs, n=20):
    """Amortized per-execution wall time (ns) of the compiled SPMD body with
    device-resident inputs. Used by test.py; not part of the grading path."""
    import time
    import jax
    from jax.sharding import Mesh, PartitionSpec, NamedSharding
    from jax.experimental.shard_map import shard_map
    import concourse.bass2jax as b2j
    import concourse.mybir as _mb

    nc = _get_nc()
    in_maps = [host_inputs(inputs, c) for c in range(8)]
    n_cores = 8
    b2j.install_neuronx_cc_hook()
    pname = nc.partition_id_tensor.name if nc.partition_id_tensor else None
    in_names, out_names, out_avals, zero_outs = [], [], [], []
    for alloc in nc.m.functions[0].allocations:
        if not isinstance(alloc, _mb.MemoryLocationSet):
            continue
        name = alloc.memorylocations[0].name
        if alloc.kind == "ExternalInput":
            if name != pname:
                in_names.append(name)
        elif alloc.kind == "ExternalOutput":
            out_names.append(name)
            shape = tuple(alloc.tensor_shape)
            dtype = _mb.dt.np(alloc.dtype)
            out_avals.append(jax.core.ShapedArray(shape, dtype))
            zero_outs.append(np.zeros(shape, dtype))
    n_params = len(in_names)
    all_in = list(in_names) + list(out_names)
    if pname is not None:
        all_in.append(pname)

    def _body(*args):
        operands = list(args)
        if pname is not None:
            operands.append(b2j.partition_id_tensor())
        return tuple(b2j._bass_exec_p.bind(
            *operands, out_avals=tuple(out_avals), in_names=tuple(all_in),
            out_names=tuple(out_names), lowering_input_output_aliases=(),
            sim_require_finite=True, sim_require_nnan=True, nc=nc))

    devices = jax.devices()[:n_cores]
    mesh = Mesh(np.asarray(devices), ("core",))
    specs = (PartitionSpec("core"),) * (n_params + len(out_names))
    fn = jax.jit(shard_map(_body, mesh=mesh, in_specs=specs,
                           out_specs=(PartitionSpec("core"),) * len(out_names),
                           check_rep=False), keep_unused=True)
    per_core = [[np.asarray(m[nm]) for nm in in_names] for m in in_maps]
    concat_in = [np.concatenate([per_core[c][i] for c in range(n_cores)])
                 for i in range(n_params)]
    concat_zero = [np.zeros((n_cores * z.shape[0], *z.shape[1:]), z.dtype)
                   for z in zero_outs]
    sh = NamedSharding(mesh, PartitionSpec("core"))
    dev_in = [jax.device_put(a, sh) for a in concat_in + concat_zero]
    out = fn(*dev_in)
    jax.block_until_ready(out)
    best = None
    for _ in range(3):
        t0 = time.time()
        for _ in range(n):
            out = fn(*dev_in)
        jax.block_until_ready(out)
        dt = (time.time() - t0) / n * 1e9
        best = dt if best is None else min(best, dt)
    return best


# revision 46
# speedup vs baseline: 1.0049x; 1.0049x over previous
"""Trainium2 Bass kernel for nn_DFlashSelfAttention (block-sparse GQA attention).

Self-contained: builds the Bass module once, shards inputs over 8 NeuronCores
(sequence-parallel), runs via run_bass_kernel_spmd, reassembles full output.
"""

import sys as _sys
for _p in ("/opt/trn_rl_repo",):
    if _p not in _sys.path:
        _sys.path.insert(0, _p)

"""Bass/Tile kernel for DFlashSelfAttention (block-diagonal causal attention).

Sharding: sequence-parallel over L (2048 -> 8 cores x 256 positions).
Attention is block-diagonal with BLOCK=16, so positions never interact
across 16-blocks; a 256-position slice (16 blocks) is fully independent.

The two big GEMMs (X@Wqkv and A@Wo) run in fp8(e4m3) DoubleRow perf mode
(2 contraction rows/cycle) with residual compensation: each operand O is
split host-side (or on-device for A) into Oh = e4m3(s*O) and
Ol = e4m3(s*O - Oh) at the SAME scale, so the three product terms
Oh*Wh + Ol*Wh + Oh*Wl accumulate directly in PSUM (lo*lo dropped).
This costs 3 DoubleRow passes = 0.75x the fp16 matmul cycles at ~1.8e-3
final relative error (fp16 everywhere gives 5.4e-4; tolerance is 2e-2).

Per-core pipeline (T = 512 rows = 2 batches x 256 positions):
  phase A: Q = X @ Wq, 3-pass fp8 over all 8 psum banks.
  phase B: KV = X @ Wkv into banks freed by Q->SBUF drains (descale 2^-17
    folded into the drain copies; V additionally scaled by 32 = SA).
  per 128-token chunk: fp16 RMS-norm+RoPE (rstd = exp(-ln(v)/2) keeps the
    ACT engine on one table set), PE pair-transposes of Q^T/K^T, then
    GROUP-BATCHED attention: per kv-head group g of 4 query heads one
    [128,512] psum tile holds mask+S for (head, qtok); one ACT exp; column
    sums via GPSIMD partition_all_reduce; one AV matmul lands at psum
    partitions (g%2)*64; normalization = per-pair DVE reciprocal + one DVE
    multiply straight from PSUM into A^T (fp16, = 32*A), then ACT cast to
    fp8 hi + DVE subtract to fp8 lo.
  emit: Y^T = Wo^T @ A^T per 256-token half (3-pass fp8 DoubleRow),
    interleaved with the chunk-2/3 attention chains; fp16 DRAM [4096, 512];
    host transposes back.

Attention math stays fp16 (1 cy/row): RMS-norm weights and the sqrt(1/8)
attention scale are folded into host-precomputed rope tables.
"""

import ml_dtypes
import numpy as np

import concourse.bass as bass
import concourse.mybir as mybir
import concourse.tile as tile
from concourse import bacc
from concourse.bass_isa import ReduceOp
from concourse.masks import make_identity

F32 = mybir.dt.float32
F16 = mybir.dt.float16
F8 = mybir.dt.float8e4
E4 = ml_dtypes.float8_e4m3

P = 128
HID = 4096
KO = HID // P          # 32 k-chunks over hidden
KP = KO // 2           # 16 DoubleRow k-pairs
T = 512                # rows per core: 2 batches x 256 positions
NM = T // P            # 4 t-chunks
NH = 16
NKV = 4
HD = 64
H2 = HD // 2
QD = NH * HD           # 1024
KVD = 2 * NKV * HD     # 512 (k 256 | v 256)
EPS = 1e-6

SX = 32.0              # hidden-states fp8 scale
SW = 4096.0            # weights fp8 scale (Wqkv, Wo)
SA = 32.0              # attention-out fp8 scale
DS = 1.0 / (SX * SW)   # qkv psum descale = 2^-17 (also = 1/(SA*SW) for emit)
DSV = DS * SA          # v drain: descale then re-scale by SA

DR = mybir.MatmulPerfMode.DoubleRow

# Q-head permutation: position p holds original head PERM[p]. Even positions
# carry heads of even kv parity groups; transposed pair-tiles then expose each
# kv group's 4 heads as one contiguous [64, 4, 128] moving operand.
PERM = [0, 4, 1, 5, 2, 6, 3, 7, 8, 12, 9, 13, 10, 14, 11, 15]


def _pin_act_tables():
    """Make every activation resolve to the natural_log_exp_and_others set so
    the act-table pass emits one load instead of ping-ponging between the
    exp-only and ln-only tables. Indices (act_func_set_id) are preserved."""
    import concourse.bacc as _bacc_mod
    from concourse import hw_specs as _hw
    real = _hw.get_activation_tables

    def pinned(arch):
        t = real(arch)
        keep = "natural_log_exp_and_others"
        if keep not in t:
            return t
        return {nm: (fns if nm == keep else set()) for nm, fns in t.items()}

    _bacc_mod.get_activation_tables = pinned
    return _bacc_mod, real


def build_nc(name="dfa"):
    _bacc_mod, _real_gat = _pin_act_tables()
    try:
        return _build_nc_inner(name)
    finally:
        _bacc_mod.get_activation_tables = _real_gat


def _build_nc_inner(name="dfa"):
    nc = bacc.Bacc(None, target_bir_lowering=False, name=name)

    # hi/lo fp8 pairs packed on dim 1 (v: 0=hi, 1=lo)
    xt = nc.dram_tensor("xt", [P, 2, KO, T], F8, kind="ExternalInput")
    wq = nc.dram_tensor("wq", [P, 2, KO, QD], F8, kind="ExternalInput")
    wkv = nc.dram_tensor("wkv", [P, 2, KO, KVD], F8, kind="ExternalInput")
    wo = nc.dram_tensor("wo", [P, 2, 32, 8, P], F8, kind="ExternalInput")
    tabs = nc.dram_tensor("tabs", [T, 4 * HD], F16, kind="ExternalInput")
    mask4 = nc.dram_tensor("mask4", [P, 2 * 4 * P], F8, kind="ExternalInput")
    identw = nc.dram_tensor("identw", [P, 2 * P], F8, kind="ExternalInput")
    yt = nc.dram_tensor("yt", [HID, T], F16, kind="ExternalOutput")

    tabs_r = tabs.rearrange("(m p) d -> p m d", p=P)
    yt_r = yt.rearrange("(mo p) t -> p mo t", p=P)

    from contextlib import ExitStack
    with tile.TileContext(nc) as tc, ExitStack() as ctx:
        consts = ctx.enter_context(tc.tile_pool(name="consts", bufs=1))
        xt_pool = ctx.enter_context(tc.tile_pool(name="xt", bufs=1))
        wstream = ctx.enter_context(tc.tile_pool(name="wstream", bufs=4))
        acts = ctx.enter_context(tc.tile_pool(name="acts", bufs=1))
        rope_tmp = ctx.enter_context(tc.tile_pool(name="rope_tmp", bufs=1))
        attn_tmp = ctx.enter_context(tc.tile_pool(name="attn_tmp", bufs=2))
        ystage = ctx.enter_context(tc.tile_pool(name="ystage", bufs=2))
        pp = ctx.enter_context(tc.tile_pool(name="pp", bufs=1, space="PSUM"))

        def ptile(shape, bank, name, dtype=F32):
            tot = 512 if dtype == F32 else 1024
            pad = list(shape)
            pad[-1] = max(1, tot // int(np.prod(shape[1:-1])))
            return pp.tile(shape, dtype, tag=f"b{bank}", name=name,
                           padded_shape=pad)

        # ---- leading DMAs: hi parts first so the hi*hi pass of k-pair 0 can
        # start after half the bytes; few large pieces — the serialized
        # ~625ns HWDGE config per DMA, not transfer time, is what delays
        # phase A ----
        # constants are engine-local — issuing them first costs the DMA
        # stream nothing and un-gates the warmup matmuls
        ident = consts.tile([P, P], F16)
        make_identity(nc, ident)
        eps_t = consts.tile([P, 1], F32)
        nc.vector.memset(eps_t, EPS)

        xt_sb = xt_pool.tile([P, 2, KO, T], F8)
        wq_b0 = wstream.tile([P, 2, 4, QD], F8, tag="wq4", bufs=3,
                             name="wq_b0")
        nc.sync.dma_start(xt_sb[:, 0:1, 0:2, :], xt[:, 0:1, 0:2, :])
        nc.sync.dma_start(wq_b0[:, 0:1, 0:2, :], wq[:, 0:1, 0:2, :])
        nc.sync.dma_start(xt_sb[:, 1:2, 0:2, :], xt[:, 1:2, 0:2, :])
        nc.sync.dma_start(wq_b0[:, 1:2, 0:2, :], wq[:, 1:2, 0:2, :])
        nc.sync.dma_start(wq_b0[:, :, 2:4, :], wq[:, :, 2:4, :])
        nc.sync.dma_start(xt_sb[:, :, 2:4, :], xt[:, :, 2:4, :])

        # wq coarse batches (bufs=4 so FIFO WAR stalls never starve phase A).
        # wkv/mask/tabs are deferred past the phase-A window: its DMA demand
        # (wq+xt+wkv) would exceed supply by ~1.1us per batch; wkv tile kb is
        # not read until b_phase, which trails phase A by >=4us
        wq_coarse = []
        wkv_tiles = []
        mask_sb = consts.tile([P, 2, 4, P], F8)
        identw_sb = consts.tile([P, 2, P], F8)
        tabs_sb = consts.tile([P, NM, 4, HD], F16)

        def wkv_load(kb):
            wkvt = wstream.tile([P, 2, 4, KVD], F8, tag="wkv", bufs=8,
                                name=f"wkv_k{kb}")
            nc.sync.dma_start(wkvt[:], wkv[:, :, kb * 4:(kb + 1) * 4, :])
            wkv_tiles.append(wkvt)

        for kb in range(1, 8):
            wqt = wstream.tile([P, 2, 4, QD], F8, tag="wq4", bufs=3,
                               name=f"wq_b{kb}")
            nc.sync.dma_start(wqt[:], wq[:, :, kb * 4:(kb + 1) * 4, :])
            wq_coarse.append(wqt)
            nc.sync.dma_start(xt_sb[:, :, kb * 4:(kb + 1) * 4, :],
                              xt[:, :, kb * 4:(kb + 1) * 4, :])
            if kb >= 3:
                wkv_load(kb - 3)
        # tabs/mask feed the rope/scores chains that now start right at the
        # head of phase B — ahead of the tail wkv tiles
        nc.sync.dma_start(
            tabs_sb[:], tabs_r.rearrange("p m (f d) -> p m f d", d=HD))
        nc.sync.dma_start(
            mask_sb[:], mask4.rearrange("p (v f q) -> p v f q", v=2, q=P))
        nc.sync.dma_start(
            identw_sb[:], identw.rearrange("p (v q) -> p v q", v=2))
        for kb in range(5, 8):
            wkv_load(kb)

        # PE warmup: anchor the p-state ramp while the first DMAs land
        warm_src = consts.tile([P, 512], F16)
        nc.vector.memset(warm_src, 0.0)
        for w in range(3):
            wps = ptile([P, 512], 0, f"warm{w}")
            nc.tensor.matmul(wps[:], warm_src[:, 0:P], warm_src[:],
                             start=True, stop=True)

        # ---- phase A: Q = X @ Wq, 3-pass fp8 DoubleRow over all 8 banks ----
        ps_a = [[ptile([P, 512], 2 * m + s, f"psa{m}_{s}")
                 for s in range(2)] for m in range(NM)]

        PASSES = ((0, 0), (1, 0), (0, 1))   # (vx, vw): hi*hi, lo*hi, hi*lo

        def a_matmuls(kp, wq_tile, jo, passes=(0, 1, 2)):
            """kp: global k-pair (0..15); wq_tile sliced at pair offset jo."""
            for m in range(NM):
                ms = slice(m * P, (m + 1) * P)
                for s in range(2):
                    ss = slice(s * 512, (s + 1) * 512)
                    for pi in passes:
                        vx, vw = PASSES[pi]
                        nc.tensor.matmul(
                            ps_a[m][s][:],
                            xt_sb[:, vx, 2 * kp:2 * kp + 2, ms],
                            wq_tile[:, vw, 2 * jo:2 * jo + 2, ss],
                            start=(kp == 0 and pi == 0),
                            stop=(kp == KP - 1 and pi == 2), perf_mode=DR)

        a_matmuls(0, wq_b0, 0, passes=(0,))    # needs only hi DMAs
        a_matmuls(0, wq_b0, 0, passes=(1,))    # + xt lo
        a_matmuls(0, wq_b0, 0, passes=(2,))    # + wq lo
        a_matmuls(1, wq_b0, 1)
        for kb in range(1, 8):
            for jo in range(2):
                a_matmuls(2 * kb + jo, wq_coarse[kb - 1], jo)

        # ---- Q drains (DVE/ACT split, descale 2^-17) + phase B + KV drains
        qn = [acts.tile([P, NH, HD], F16, tag=f"qn{m}", name=f"qn{m}")
              for m in range(NM)]

        def qn_copy(m):
            nc.vector.tensor_scalar_mul(
                qn[m][:, 0:8, :],
                ps_a[m][0][:].rearrange("p (h d) -> p h d", d=HD), DS)
            nc.scalar.activation(
                qn[m][:, 8:16, :],
                ps_a[m][1][:].rearrange("p (h d) -> p h d", d=HD),
                mybir.ActivationFunctionType.Copy, scale=DS)

        ps_b = []
        kn = []
        v_sb = []

        def b_phase(m):
            pb = ptile([P, KVD], 6 + m % 2, f"psb{m}")
            ps_b.append(pb)
            ms = slice(m * P, (m + 1) * P)
            ni = 0
            for kb in range(8):
                for jo in range(2):
                    kp = 2 * kb + jo
                    for pi, (vx, vw) in enumerate(((0, 0), (1, 0), (0, 1))):
                        nc.tensor.matmul(
                            pb[:], xt_sb[:, vx, 2 * kp:2 * kp + 2, ms],
                            wkv_tiles[kb][:, vw, 2 * jo:2 * jo + 2, :],
                            start=(ni == 0), stop=(ni == 3 * KP - 1),
                            perf_mode=DR)
                        ni += 1

        def kv_drain(m):
            knt = acts.tile([P, NKV, HD], F16, tag=f"kn{m}", name=f"kn{m}")
            nc.vector.tensor_scalar_mul(
                knt[:], ps_b[m][:, 0:256].rearrange("p (h d) -> p h d", d=HD),
                DS)
            kn.append(knt)
            vt = acts.tile([P, 256], F16, tag=f"v{m}", name=f"v{m}")
            nc.scalar.activation(vt[:], ps_b[m][:, 256:512],
                                 mybir.ActivationFunctionType.Copy, scale=DSV)
            v_sb.append(vt)

        qn_copy(3)      # frees banks 6/7 for the b_phase rotation first
        qn_copy(0)
        b_phase(0)
        kv_drain(0)
        qn_copy(1)
        qn_copy(2)

        # ---- rope helpers (fp16 math; rstd via ln/exp on ACT) ----
        def stats(src, nh, m, tag):
            sq = rope_tmp.tile([P, nh, HD], F16, tag=f"sq{nh}", bufs=1,
                               name=f"sq_{tag}")
            nc.vector.tensor_mul(sq[:], src[:], src[:])
            ssq = rope_tmp.tile([P, nh], F32, tag=f"ssq{nh}", bufs=2,
                                name=f"ssq_{tag}")
            nc.vector.reduce_sum(ssq[:], sq[:], axis=mybir.AxisListType.X)
            lnv = rope_tmp.tile([P, nh], F32, tag=f"lnv{nh}", bufs=2,
                                name=f"lnv_{tag}")
            nc.scalar.activation(lnv[:], ssq[:],
                                 mybir.ActivationFunctionType.Ln,
                                 bias=eps_t[:], scale=1.0 / HD)
            rstd = rope_tmp.tile([P, nh], F16, tag=f"rstd{nh}", bufs=2,
                                 name=f"rstd_{tag}")
            nc.scalar.activation(rstd[:], lnv[:],
                                 mybir.ActivationFunctionType.Exp, scale=-0.5)
            return rstd

        def rope_apply(src, rstd, m, nh, cf, sf, tag):
            qn2 = rope_tmp.tile([P, nh, HD], F16, tag=f"qn2_{nh}", bufs=2,
                                name=f"qn2_{tag}")
            nc.vector.tensor_mul(qn2[:], src[:],
                                 rstd[:, :, None].to_broadcast((P, nh, HD)))
            ctab = tabs_sb[:, m, cf, :]
            stab = tabs_sb[:, m, sf, :]
            o1 = rope_tmp.tile([P, nh, HD], F16, tag=f"o1_{nh}", bufs=1,
                               name=f"o1_{tag}")
            nc.vector.tensor_mul(o1[:], qn2[:],
                                 ctab[:, None, :].to_broadcast((P, nh, HD)))
            o2 = rope_tmp.tile([P, nh, HD], F16, tag=f"o2_{nh}", bufs=1,
                               name=f"o2_{tag}")
            nc.vector.tensor_mul(
                o2[:, :, 0:H2], qn2[:, :, H2:HD],
                stab[:, None, 0:H2].to_broadcast((P, nh, H2)))
            nc.vector.tensor_mul(
                o2[:, :, H2:HD], qn2[:, :, 0:H2],
                stab[:, None, H2:HD].to_broadcast((P, nh, H2)))
            out = rope_tmp.tile([P, nh * HD], F16, tag=f"ro_{nh}", bufs=2,
                                name=f"ro_{tag}")
            nc.vector.tensor_add(out[:], o1[:].rearrange("p h d -> p (h d)"),
                                 o2[:].rearrange("p h d -> p (h d)"))
            return out

        krT = {}
        qrT = {}
        rope_out_k = {}
        rope_out_q = {}

        def rope_k(m):
            """DVE/ACT-only: rms-norm + rope for k of chunk m (needs kv m)."""
            rstd_k = stats(kn[m], NKV, m, f"k{m}")
            rope_out_k[m] = rope_apply(kn[m], rstd_k, m, NKV, 2, 3, f"k{m}")

        def rope_q(m):
            """Needs only qn (phase-A drains) + tabs — hoistable early."""
            rstd_q = stats(qn[m], NH, m, f"q{m}")
            rope_out_q[m] = rope_apply(qn[m], rstd_q, m, NH, 0, 1, f"q{m}")

        def rope_kq(m):
            rope_k(m)
            rope_q(m)

        def trans_kq(m):
            """PE transposes + ACT psum drains for chunk m (needs
            rope_kq(m)); XBAR DMA transposes were tried and regress — their
            transfers queue behind 1.5us bulk weight DMAs on the shared DMA
            engines right when the S matmuls need them."""
            kr = rope_out_k[m]
            qr = rope_out_q[m]
            kps = ptile([P, 2, P], 4, f"krT_ps{m}", F16)
            for j in range(2):
                nc.tensor.matmul(kps[:, j, :], kr[:, j * P:(j + 1) * P],
                                 ident[:], is_transpose=True)
            kt = acts.tile([P, 2, P], F16, tag=f"krT{m}", name=f"krT{m}")
            nc.scalar.copy(kt[:], kps[:])
            krT[m] = kt
            qps = ptile([P, 8, P], 5, f"qrT_ps{m}", F16)
            for j in range(8):
                nc.tensor.matmul(qps[:, j, :],
                                 qr[:, 2 * j * HD:(2 * j + 2) * HD],
                                 ident[:], is_transpose=True)
            qt = acts.tile([P, 8, P], F16, tag=f"qrT{m}", name=f"qrT{m}")
            nc.scalar.copy(qt[:], qps[:])
            qrT[m] = qt

        # A^T accumulator (= SA * A): partition (g%2)*64+d, col c=(g//2)*4+i,
        # token t. at16 fp16 + on-device fp8 hi/lo split for the emit GEMM.
        at16 = acts.tile([P, 8, T], F16, tag="at16")
        at8h = acts.tile([P, 8, T], F8, tag="at8h")
        at8l = acts.tile([P, 8, T], F8, tag="at8l")

        # ---- group-batched attention, split into a scores half and an AV
        # half so other PE work (phase B tail, transposes, emits) can sit
        # between them and hide the exp->gpsimd chain latency ----
        attn_state = {}

        def attn_scores(m):
            sums = attn_tmp.tile([P, 4, 512], F32, tag="sums", bufs=1,
                                 name=f"sums{m}")
            # pair layout: partition half (g%2)*64 of column gp holds group
            # g's reciprocal, matching o_ps[gp]'s layout so one [128,512]
            # multiply normalizes both groups of a pair at once
            rcp = attn_tmp.tile([P, 2, 512], F32, tag="rcp", bufs=1,
                                name=f"rcp{m}")
            ests = []
            attn_state[m] = (rcp, {}, sums, ests)
            for g in range(4):
                s_ps = ptile([P, 512], g % 2, f"s{m}_{g}")
                nc.tensor.matmul(s_ps[:], identw_sb[:],
                                 mask_sb[:].rearrange("p v f q -> p v (f q)"),
                                 start=True, stop=False, perf_mode=DR)
                base = (g % 2) * HD
                gp = g // 2
                nc.tensor.matmul(s_ps[:],
                                 krT[m][base:base + HD, gp, :],
                                 qrT[m][base:base + HD, 4 * gp:4 * gp + 4, :],
                                 start=False, stop=True)
                est = attn_tmp.tile([P, 4, P], F16, tag="est", bufs=4,
                                    name=f"est{m}_{g}")
                nc.scalar.activation(est[:], s_ps[:],
                                     mybir.ActivationFunctionType.Exp)
                nc.gpsimd.partition_all_reduce(sums[:, g, :], est[:],
                                               channels=P,
                                               reduce_op=ReduceOp.add)
                ests.append(est)

        def attn_avs(m):
            rcp, o_ps, sums, ests = attn_state[m]
            for g in range(4):
                base = (g % 2) * HD
                gp = g // 2
                if gp not in o_ps:
                    o_ps[gp] = ptile([P, 512], 2 + gp, f"o{m}_{gp}")
                nc.tensor.matmul(o_ps[gp][base:base + HD, :],
                                 v_sb[m][:, g * HD:(g + 1) * HD], ests[g][:],
                                 start=True, stop=True)
                if g % 2 == 1:
                    # per-pair reciprocals into the pair layout; lane-locked
                    # halves keep the DVE queue moving
                    nc.vector.reciprocal(rcp[0:HD, gp, :],
                                         sums[0:HD, g - 1, :])
                    nc.vector.reciprocal(rcp[HD:P, gp, :],
                                         sums[HD:P, g, :])

        def attn_core(m):
            attn_scores(m)
            attn_avs(m)

        def attn_norm(m):
            rcp, o_ps, _, _ = attn_state[m]
            ms = slice(m * P, (m + 1) * P)
            for gp in range(2):
                # one multiply normalizes both groups of the pair: partition
                # halves of o_ps/rcp line up with at16's (g%2) layout
                nc.vector.tensor_mul(
                    at16[:, 4 * gp:4 * gp + 4, ms],
                    o_ps[gp][:].rearrange("p (i t) -> p i t", t=P),
                    rcp[:, gp, :].rearrange("p (i t) -> p i t", t=P))
            # fp8 hi/lo split for the emit GEMM: cast on DVE, residual on
            # GPSIMD — keeps the ACT queue pure exp (its latency releases the
            # S psum banks) and the softmax-critical Pool sums unobstructed
            nc.vector.tensor_copy(at8h[:, :, ms], at16[:, :, ms])
            nc.gpsimd.tensor_sub(at8l[:, 0:4, ms], at16[:, 0:4, ms],
                                 at8h[:, 0:4, ms])
            nc.vector.tensor_sub(at8l[:, 4:8, ms], at16[:, 4:8, ms],
                                 at8h[:, 4:8, ms])

        # ---- emit: Y^T = Wo^T @ A^T (3-pass fp8 DoubleRow); two mo's pair
        # up per psum bank so one [P,512] copy drains them and the WAR
        # pipeline is 4 mo's deep ----
        def emit_mb(half, mb, bank, split_tail=False, wo_tile=None):
            c0 = half * 256
            tsl = slice(c0, c0 + 256)
            if wo_tile is not None:
                wo_m = wo_tile
            else:
                # mb 0/1 tiles stay resident ("wo01") for the final half-1
                # emits so the tail has no wo DMAs in front of its yt writes
                tag, bufs = ("wo01", 1) if mb < 1 else ("wo", 3)
                wo_m = wstream.tile([P, 2, 4, 8, P], F8, tag=tag, bufs=bufs,
                                    name=f"wo_m{half}_{mb}")
                nc.sync.dma_start(wo_m[:], wo[:, :, mb * 4:(mb + 1) * 4, :, :])
            ys = ystage.tile([P, 4, 256], F16, tag="ys", name="ys")
            for pair in range(2):
                ps = ptile([P, 2, 256], bank[pair], f"ps_y{half}_{mb}_{pair}")
                for sub2 in range(2):
                    sub = pair * 2 + sub2
                    # at8h-only passes first, at8l pass last, all within ONE
                    # contiguous start..stop group (interleaving OPEN groups
                    # in a bank silently zeroes partial sums on hw)
                    ni = 0
                    for (va, vw) in ((0, 0), (0, 1), (1, 0)):
                        at_op = at8h if va == 0 else at8l
                        for u in range(4):
                            nc.tensor.matmul(
                                ps[:, sub2, :],
                                wo_m[:, vw, sub, 2 * u:2 * u + 2, :],
                                at_op[:, 2 * u:2 * u + 2, tsl],
                                start=(ni == 0), stop=(ni == 11),
                                perf_mode=DR)
                            ni += 1
                # alternate copy engines by (mb+pair) parity so consecutive
                # same-bank drains never queue behind each other
                on_dve = (mb + pair) % 2 == 0
                ys_dst = ys[:, 2 * pair:2 * pair + 2, :] \
                    .rearrange("p i t -> p (i t)")
                ps_src = ps[:].rearrange("p i t -> p (i t)")
                if on_dve:
                    nc.vector.tensor_scalar_mul(ys_dst, ps_src, DS)
                else:
                    nc.scalar.activation(ys_dst, ps_src,
                                         mybir.ActivationFunctionType.Copy,
                                         scale=DS)
                if split_tail:
                    nc.sync.dma_start(
                        yt_r[:, mb * 4 + 2 * pair:mb * 4 + 2 * pair + 2,
                             c0:c0 + 256],
                        ys[:, 2 * pair:2 * pair + 2, :])
            if not split_tail:
                nc.sync.dma_start(yt_r[:, mb * 4:(mb + 1) * 4, c0:c0 + 256],
                                  ys[:])
            return wo_m

        # ---- schedule: rope chains and transposes overlap the tail of
        # phase B; attention starts the moment B's last matmul retires;
        # each chunk's normalize+fp8-split follows its core so half-0 emits
        # (tokens 0..255 = chunks 0,1) can interleave with chunk 2/3
        # attention and fill the PE gaps of the latency-bound softmax chain
        # attention chunks pipeline INTO phase B: rope chains (DVE/ACT)
        # issue one b_phase ahead of their PE transposes, scores' exp/sums
        # latency hides behind the next b_phase's matmuls, and the emit
        # stream starts right after the last AV chain
        rope_kq(0)
        b_phase(1)
        kv_drain(1)
        rope_kq(1)
        trans_kq(0)
        b_phase(2)
        kv_drain(2)
        rope_kq(2)
        trans_kq(1)
        attn_scores(0)
        rope_q(3)        # q-rope of chunk 3 needs no phase-B data: its DVE
                         # chain runs under b3 instead of crowding the
                         # norm-critical window after it
        b_phase(3)       # fills the exp->gpsimd latency of chunk 0
        kv_drain(3)
        attn_avs(0)
        attn_scores(1)
        trans_kq(2)      # after scores(1): its ACT copies stay behind S1's
                         # bank-releasing exps
        attn_norm(0)     # DVE: frees o_ps banks 2/3 before rope-k(3) queues
        rope_k(3)
        attn_avs(1)
        attn_scores(2)
        trans_kq(3)
        attn_norm(1)
        attn_avs(2)
        attn_scores(3)
        attn_norm(2)
        wo_01 = emit_mb(0, 0, (6, 7))
        attn_avs(3)
        emit_mb(0, 1, (4, 5))
        attn_norm(3)
        # late mb stages cover both token halves from one wo load (wo would
        # otherwise be streamed twice: ~15us of DMA traffic saved); half-0
        # rotates (6,7)/(4,5) and half-1 (0,1)/(2,3) so four banks pipeline
        H1B = [(0, 1), (2, 3)]
        H0B = [(6, 7), (4, 5)]
        wo_t11 = None
        for mb in range(2, 8):
            wo_t = emit_mb(0, mb, H0B[mb % 2])
            if mb == 7:
                # prefetch the mb1 reload while the mb7 emits run so the
                # tail has no wo DMA in front of its yt writes
                wo_t11 = wstream.tile([P, 2, 4, 8, P], F8, tag="wo", bufs=3,
                                      name="wo_m1_reload")
                nc.sync.dma_start(wo_t11[:], wo[:, :, 4:8, :, :])
            emit_mb(1, mb, H1B[mb % 2], wo_tile=wo_t)
        emit_mb(1, 0, (0, 1), split_tail=True, wo_tile=wo_01)
        emit_mb(1, 1, (2, 3), split_tail=True, wo_tile=wo_t11)

    nc.finalize()
    return nc


def _split8(a, s):
    """fp8 hi/lo residual split at common scale s: a*s ~= hi + lo."""
    hi = (a * s).astype(E4)
    lo = (a * s - hi.astype(np.float32)).astype(E4)
    return hi, lo


def host_inputs(inputs, core):
    """Build the per-core DRAM input map from full problem inputs."""
    hs = np.asarray(inputs["hidden_states"], np.float32)
    am = np.asarray(inputs["attention_mask"], np.float32)
    cos = np.asarray(inputs["cos"], np.float32)
    sin = np.asarray(inputs["sin"], np.float32)
    Wqkv = np.asarray(inputs["Wqkv"], np.float32)
    Wo = np.asarray(inputs["Wo"], np.float32)
    qw = np.asarray(inputs["q_norm_w"], np.float32)
    kw = np.asarray(inputs["k_norm_w"], np.float32)

    LS = 256
    ls = slice(core * LS, (core + 1) * LS)
    X = hs[:, ls, :].reshape(T, HID)
    xt_f = np.ascontiguousarray(X.T)                      # [HID, T]
    xh, xl = _split8(xt_f, SX)
    # pack [HID, T] -> [P, 2, KO, T]
    xt8 = np.stack([xh.reshape(KO, P, T), xl.reshape(KO, P, T)], axis=0) \
        .transpose(2, 0, 1, 3)

    cos_c = cos[:, ls, :].reshape(T, HD)
    sin_c = sin[:, ls, :].reshape(T, HD)
    sq = float(HD) ** -0.25  # sqrt(1/sqrt(HD)) = sqrt(1/8)
    swap = np.concatenate([np.arange(32, 64), np.arange(0, 32)])
    sign = np.concatenate([-np.ones(32, np.float32), np.ones(32, np.float32)])

    tabs = np.empty((T, 4, HD), np.float32)
    tabs[:, 0, :] = cos_c * qw[None, :] * sq
    tabs[:, 1, :] = sin_c * qw[swap][None, :] * sign[None, :] * sq
    tabs[:, 2, :] = cos_c * kw[None, :] * sq
    tabs[:, 3, :] = sin_c * kw[swap][None, :] * sign[None, :] * sq

    # fp8 DoubleRow mask: v0 holds mask/16 (0 or -240), v1 zeros; the
    # stationary identw v0 is 16*I so the product restores -3840 (exp -> 0)
    maskT8 = np.where(am[0, 0, :P, :P].T < -1.0, -240.0, 0.0).astype(E4)
    mask8 = np.zeros((P, 2, 4, P), E4)
    mask8[:, 0, :, :] = np.broadcast_to(maskT8[:, None, :], (P, 4, P))
    identw = np.zeros((P, 2, P), E4)
    identw[:, 0, :] = (np.eye(P, dtype=np.float32) * 16.0).astype(E4)

    wq_f = np.ascontiguousarray(
        Wqkv[:, :QD].reshape(HID, NH, HD)[:, PERM, :].reshape(HID, QD))
    wqh, wql = _split8(wq_f, SW)
    wq8 = np.stack([wqh.reshape(KO, P, QD), wql.reshape(KO, P, QD)],
                   axis=0).transpose(2, 0, 1, 3)

    wkv_f = Wqkv[:, QD:]
    wkh, wkl = _split8(wkv_f, SW)
    wkv8 = np.stack([wkh.reshape(KO, P, KVD), wkl.reshape(KO, P, KVD)],
                    axis=0).transpose(2, 0, 1, 3)

    # wo[p=(par,d), mo, c, j] = Wo[h(c,par)*64+d, mo*128+j]
    woh = Wo.reshape(NH, HD, 32, P)
    wo_np = np.empty((P, 32, 8, P), np.float32)
    for par in range(2):
        for c in range(8):
            h = 8 * (c // 4) + 4 * par + (c % 4)
            wo_np[par * 64:(par + 1) * 64, :, c, :] = woh[h]
    woh8, wol8 = _split8(wo_np, SW)
    wo8 = np.stack([woh8, wol8], axis=1)                   # [P, 2, 32, 8, P]

    m = {
        "xt": np.ascontiguousarray(xt8),
        "tabs": np.ascontiguousarray(tabs.reshape(T, 4 * HD)).astype(np.float16),
        "wq": np.ascontiguousarray(wq8),
        "wkv": np.ascontiguousarray(wkv8),
        "wo": np.ascontiguousarray(wo8),
        "mask4": np.ascontiguousarray(mask8.reshape(P, 2 * 4 * P)),
        "identw": np.ascontiguousarray(identw.reshape(P, 2 * P)),
    }
    return m


def assemble_output(yts):
    """yts: list of 8 [4096, 512] fp16 arrays -> [2, 2048, 4096] f32."""
    out = np.empty((2, 2048, HID), np.float32)
    for c, yt_ in enumerate(yts):
        sl = yt_.astype(np.float32).T.reshape(2, 256, HID)
        out[:, c * 256:(c + 1) * 256, :] = sl
    return out


_NC_CACHE = {}


def _get_nc():
    if "nc" not in _NC_CACHE:
        _NC_CACHE["nc"] = build_nc()
    return _NC_CACHE["nc"]


def _run(inputs, trace=False):
    from concourse.bass_utils import run_bass_kernel_spmd
    nc = _get_nc()
    in_maps = [host_inputs(inputs, c) for c in range(8)]
    res = run_bass_kernel_spmd(nc, in_maps, core_ids=list(range(8)),
                               trace=trace)
    out = assemble_output([res.results[c]["yt"] for c in range(8)])
    return out, res


def kernel(**inputs):
    out, _ = _run(inputs, trace=False)
    if not np.isfinite(out).all():
        # transient first-execution flake seen once on device; retry
        out, _ = _run(inputs, trace=False)
    return out


def _timed_runs(inputs, n=20):
    """Amortized per-execution wall time (ns) of the compiled SPMD body with
    device-resident inputs. Used by test.py; not part of the grading path."""
    import time
    import jax
    from jax.sharding import Mesh, PartitionSpec, NamedSharding
    from jax.experimental.shard_map import shard_map
    import concourse.bass2jax as b2j
    import concourse.mybir as _mb

    nc = _get_nc()
    in_maps = [host_inputs(inputs, c) for c in range(8)]
    n_cores = 8
    b2j.install_neuronx_cc_hook()
    pname = nc.partition_id_tensor.name if nc.partition_id_tensor else None
    in_names, out_names, out_avals, zero_outs = [], [], [], []
    for alloc in nc.m.functions[0].allocations:
        if not isinstance(alloc, _mb.MemoryLocationSet):
            continue
        name = alloc.memorylocations[0].name
        if alloc.kind == "ExternalInput":
            if name != pname:
                in_names.append(name)
        elif alloc.kind == "ExternalOutput":
            out_names.append(name)
            shape = tuple(alloc.tensor_shape)
            dtype = _mb.dt.np(alloc.dtype)
            out_avals.append(jax.core.ShapedArray(shape, dtype))
            zero_outs.append(np.zeros(shape, dtype))
    n_params = len(in_names)
    all_in = list(in_names) + list(out_names)
    if pname is not None:
        all_in.append(pname)

    def _body(*args):
        operands = list(args)
        if pname is not None:
            operands.append(b2j.partition_id_tensor())
        return tuple(b2j._bass_exec_p.bind(
            *operands, out_avals=tuple(out_avals), in_names=tuple(all_in),
            out_names=tuple(out_names), lowering_input_output_aliases=(),
            sim_require_finite=True, sim_require_nnan=True, nc=nc))

    devices = jax.devices()[:n_cores]
    mesh = Mesh(np.asarray(devices), ("core",))
    specs = (PartitionSpec("core"),) * (n_params + len(out_names))
    fn = jax.jit(shard_map(_body, mesh=mesh, in_specs=specs,
                           out_specs=(PartitionSpec("core"),) * len(out_names),
                           check_rep=False), keep_unused=True)
    per_core = [[np.asarray(m[nm]) for nm in in_names] for m in in_maps]
    concat_in = [np.concatenate([per_core[c][i] for c in range(n_cores)])
                 for i in range(n_params)]
    concat_zero = [np.zeros((n_cores * z.shape[0], *z.shape[1:]), z.dtype)
                   for z in zero_outs]
    sh = NamedSharding(mesh, PartitionSpec("core"))
    dev_in = [jax.device_put(a, sh) for a in concat_in + concat_zero]
    out = fn(*dev_in)
    jax.block_until_ready(out)
    best = None
    for _ in range(3):
        t0 = time.time()
        for _ in range(n):
            out = fn(*dev_in)
        jax.block_until_ready(out)
        dt = (time.time() - t0) / n * 1e9
        best = dt if best is None else min(best, dt)
    return best


# revision 47
# speedup vs baseline: 1.0090x; 1.0041x over previous
"""Trainium2 Bass kernel for nn_DFlashSelfAttention (block-sparse GQA attention).

Self-contained: builds the Bass module once, shards inputs over 8 NeuronCores
(sequence-parallel), runs via run_bass_kernel_spmd, reassembles full output.
"""

import sys as _sys
for _p in ("/opt/trn_rl_repo",):
    if _p not in _sys.path:
        _sys.path.insert(0, _p)

"""Bass/Tile kernel for DFlashSelfAttention (block-diagonal causal attention).

Sharding: sequence-parallel over L (2048 -> 8 cores x 256 positions).
Attention is block-diagonal with BLOCK=16, so positions never interact
across 16-blocks; a 256-position slice (16 blocks) is fully independent.

The two big GEMMs (X@Wqkv and A@Wo) run in fp8(e4m3) DoubleRow perf mode
(2 contraction rows/cycle) with residual compensation: each operand O is
split host-side (or on-device for A) into Oh = e4m3(s*O) and
Ol = e4m3(s*O - Oh) at the SAME scale, so the three product terms
Oh*Wh + Ol*Wh + Oh*Wl accumulate directly in PSUM (lo*lo dropped).
This costs 3 DoubleRow passes = 0.75x the fp16 matmul cycles at ~1.8e-3
final relative error (fp16 everywhere gives 5.4e-4; tolerance is 2e-2).

Per-core pipeline (T = 512 rows = 2 batches x 256 positions):
  phase A: Q = X @ Wq, 3-pass fp8 over all 8 psum banks.
  phase B: KV = X @ Wkv into banks freed by Q->SBUF drains (descale 2^-17
    folded into the drain copies; V additionally scaled by 32 = SA).
  per 128-token chunk: fp16 RMS-norm+RoPE (rstd = exp(-ln(v)/2) keeps the
    ACT engine on one table set), PE pair-transposes of Q^T/K^T, then
    GROUP-BATCHED attention: per kv-head group g of 4 query heads one
    [128,512] psum tile holds mask+S for (head, qtok); one ACT exp; column
    sums via GPSIMD partition_all_reduce; one AV matmul lands at psum
    partitions (g%2)*64; normalization = per-pair DVE reciprocal + one DVE
    multiply straight from PSUM into A^T (fp16, = 32*A), then ACT cast to
    fp8 hi + DVE subtract to fp8 lo.
  emit: Y^T = Wo^T @ A^T per 256-token half (3-pass fp8 DoubleRow),
    interleaved with the chunk-2/3 attention chains; fp16 DRAM [4096, 512];
    host transposes back.

Attention math stays fp16 (1 cy/row): RMS-norm weights and the sqrt(1/8)
attention scale are folded into host-precomputed rope tables.
"""

import ml_dtypes
import numpy as np

import concourse.bass as bass
import concourse.mybir as mybir
import concourse.tile as tile
from concourse import bacc
from concourse.bass_isa import ReduceOp
from concourse.masks import make_identity

F32 = mybir.dt.float32
F16 = mybir.dt.float16
F8 = mybir.dt.float8e4
E4 = ml_dtypes.float8_e4m3

P = 128
HID = 4096
KO = HID // P          # 32 k-chunks over hidden
KP = KO // 2           # 16 DoubleRow k-pairs
T = 512                # rows per core: 2 batches x 256 positions
NM = T // P            # 4 t-chunks
NH = 16
NKV = 4
HD = 64
H2 = HD // 2
QD = NH * HD           # 1024
KVD = 2 * NKV * HD     # 512 (k 256 | v 256)
EPS = 1e-6

SX = 32.0              # hidden-states fp8 scale
SW = 4096.0            # weights fp8 scale (Wqkv, Wo)
SA = 32.0              # attention-out fp8 scale
DS = 1.0 / (SX * SW)   # qkv psum descale = 2^-17 (also = 1/(SA*SW) for emit)
DSV = DS * SA          # v drain: descale then re-scale by SA

DR = mybir.MatmulPerfMode.DoubleRow

# Q-head permutation: position p holds original head PERM[p]. Even positions
# carry heads of even kv parity groups; transposed pair-tiles then expose each
# kv group's 4 heads as one contiguous [64, 4, 128] moving operand.
PERM = [0, 4, 1, 5, 2, 6, 3, 7, 8, 12, 9, 13, 10, 14, 11, 15]


def _pin_act_tables():
    """Make every activation resolve to the natural_log_exp_and_others set so
    the act-table pass emits one load instead of ping-ponging between the
    exp-only and ln-only tables. Indices (act_func_set_id) are preserved."""
    import concourse.bacc as _bacc_mod
    from concourse import hw_specs as _hw
    real = _hw.get_activation_tables

    def pinned(arch):
        t = real(arch)
        keep = "natural_log_exp_and_others"
        if keep not in t:
            return t
        return {nm: (fns if nm == keep else set()) for nm, fns in t.items()}

    _bacc_mod.get_activation_tables = pinned
    return _bacc_mod, real


def build_nc(name="dfa"):
    _bacc_mod, _real_gat = _pin_act_tables()
    try:
        return _build_nc_inner(name)
    finally:
        _bacc_mod.get_activation_tables = _real_gat


def _build_nc_inner(name="dfa"):
    nc = bacc.Bacc(None, target_bir_lowering=False, name=name)

    # hi/lo fp8 pairs packed on dim 1 (v: 0=hi, 1=lo)
    xt = nc.dram_tensor("xt", [P, 2, KO, T], F8, kind="ExternalInput")
    wq = nc.dram_tensor("wq", [P, 2, KO, QD], F8, kind="ExternalInput")
    wkv = nc.dram_tensor("wkv", [P, 2, KO, KVD], F8, kind="ExternalInput")
    wo = nc.dram_tensor("wo", [P, 2, 32, 8, P], F8, kind="ExternalInput")
    tabs = nc.dram_tensor("tabs", [T, 4 * HD], F16, kind="ExternalInput")
    mask4 = nc.dram_tensor("mask4", [P, 2 * 4 * P], F8, kind="ExternalInput")
    identw = nc.dram_tensor("identw", [P, 2 * P], F8, kind="ExternalInput")
    yt = nc.dram_tensor("yt", [HID, T], F16, kind="ExternalOutput")

    tabs_r = tabs.rearrange("(m p) d -> p m d", p=P)
    yt_r = yt.rearrange("(mo p) t -> p mo t", p=P)

    from contextlib import ExitStack
    with tile.TileContext(nc) as tc, ExitStack() as ctx:
        consts = ctx.enter_context(tc.tile_pool(name="consts", bufs=1))
        xt_pool = ctx.enter_context(tc.tile_pool(name="xt", bufs=1))
        wstream = ctx.enter_context(tc.tile_pool(name="wstream", bufs=4))
        acts = ctx.enter_context(tc.tile_pool(name="acts", bufs=1))
        rope_tmp = ctx.enter_context(tc.tile_pool(name="rope_tmp", bufs=1))
        attn_tmp = ctx.enter_context(tc.tile_pool(name="attn_tmp", bufs=2))
        ystage = ctx.enter_context(tc.tile_pool(name="ystage", bufs=2))
        pp = ctx.enter_context(tc.tile_pool(name="pp", bufs=1, space="PSUM"))

        def ptile(shape, bank, name, dtype=F32):
            tot = 512 if dtype == F32 else 1024
            pad = list(shape)
            pad[-1] = max(1, tot // int(np.prod(shape[1:-1])))
            return pp.tile(shape, dtype, tag=f"b{bank}", name=name,
                           padded_shape=pad)

        # ---- leading DMAs: hi parts first so the hi*hi pass of k-pair 0 can
        # start after half the bytes; few large pieces — the serialized
        # ~625ns HWDGE config per DMA, not transfer time, is what delays
        # phase A ----
        # constants are engine-local — issuing them first costs the DMA
        # stream nothing and un-gates the warmup matmuls
        ident = consts.tile([P, P], F16)
        make_identity(nc, ident)
        eps_t = consts.tile([P, 1], F32)
        nc.vector.memset(eps_t, EPS)

        xt_sb = xt_pool.tile([P, 2, KO, T], F8)
        wq_b0 = wstream.tile([P, 2, 4, QD], F8, tag="wq4", bufs=3,
                             name="wq_b0")
        nc.sync.dma_start(xt_sb[:, 0:1, 0:2, :], xt[:, 0:1, 0:2, :])
        nc.sync.dma_start(wq_b0[:, 0:1, 0:2, :], wq[:, 0:1, 0:2, :])
        nc.sync.dma_start(xt_sb[:, 1:2, 0:2, :], xt[:, 1:2, 0:2, :])
        nc.sync.dma_start(wq_b0[:, 1:2, 0:2, :], wq[:, 1:2, 0:2, :])
        nc.sync.dma_start(wq_b0[:, :, 2:4, :], wq[:, :, 2:4, :])
        nc.sync.dma_start(xt_sb[:, :, 2:4, :], xt[:, :, 2:4, :])

        # wq coarse batches (bufs=4 so FIFO WAR stalls never starve phase A).
        # wkv/mask/tabs are deferred past the phase-A window: its DMA demand
        # (wq+xt+wkv) would exceed supply by ~1.1us per batch; wkv tile kb is
        # not read until b_phase, which trails phase A by >=4us
        wq_coarse = []
        wkv_tiles = []
        mask_sb = consts.tile([P, 2, 4, P], F8)
        identw_sb = consts.tile([P, 2, P], F8)
        tabs_sb = consts.tile([P, NM, 4, HD], F16)

        def wkv_load(kb):
            wkvt = wstream.tile([P, 2, 4, KVD], F8, tag="wkv", bufs=8,
                                name=f"wkv_k{kb}")
            nc.sync.dma_start(wkvt[:], wkv[:, :, kb * 4:(kb + 1) * 4, :])
            wkv_tiles.append(wkvt)

        for kb in range(1, 8):
            wqt = wstream.tile([P, 2, 4, QD], F8, tag="wq4", bufs=3,
                               name=f"wq_b{kb}")
            nc.sync.dma_start(wqt[:], wq[:, :, kb * 4:(kb + 1) * 4, :])
            wq_coarse.append(wqt)
            nc.sync.dma_start(xt_sb[:, :, kb * 4:(kb + 1) * 4, :],
                              xt[:, :, kb * 4:(kb + 1) * 4, :])
            if kb >= 3:
                wkv_load(kb - 3)
        # tabs/mask feed the rope/scores chains that now start right at the
        # head of phase B — ahead of the tail wkv tiles
        nc.sync.dma_start(
            tabs_sb[:], tabs_r.rearrange("p m (f d) -> p m f d", d=HD))
        nc.sync.dma_start(
            mask_sb[:], mask4.rearrange("p (v f q) -> p v f q", v=2, q=P))
        nc.sync.dma_start(
            identw_sb[:], identw.rearrange("p (v q) -> p v q", v=2))
        for kb in range(5, 8):
            wkv_load(kb)

        # PE warmup: anchor the p-state ramp while the first DMAs land
        warm_src = consts.tile([P, 512], F16)
        nc.vector.memset(warm_src, 0.0)
        for w in range(3):
            wps = ptile([P, 512], 0, f"warm{w}")
            nc.tensor.matmul(wps[:], warm_src[:, 0:P], warm_src[:],
                             start=True, stop=True)

        # ---- phase A: Q = X @ Wq, 3-pass fp8 DoubleRow over all 8 banks ----
        ps_a = [[ptile([P, 512], 2 * m + s, f"psa{m}_{s}")
                 for s in range(2)] for m in range(NM)]

        PASSES = ((0, 0), (1, 0), (0, 1))   # (vx, vw): hi*hi, lo*hi, hi*lo

        def a_matmuls(kp, wq_tile, jo, passes=(0, 1, 2)):
            """kp: global k-pair (0..15); wq_tile sliced at pair offset jo."""
            for m in range(NM):
                ms = slice(m * P, (m + 1) * P)
                for s in range(2):
                    ss = slice(s * 512, (s + 1) * 512)
                    for pi in passes:
                        vx, vw = PASSES[pi]
                        nc.tensor.matmul(
                            ps_a[m][s][:],
                            xt_sb[:, vx, 2 * kp:2 * kp + 2, ms],
                            wq_tile[:, vw, 2 * jo:2 * jo + 2, ss],
                            start=(kp == 0 and pi == 0),
                            stop=(kp == KP - 1 and pi == 2), perf_mode=DR)

        a_matmuls(0, wq_b0, 0, passes=(0,))    # needs only hi DMAs
        a_matmuls(0, wq_b0, 0, passes=(1,))    # + xt lo
        a_matmuls(0, wq_b0, 0, passes=(2,))    # + wq lo
        a_matmuls(1, wq_b0, 1)
        for kb in range(1, 8):
            for jo in range(2):
                a_matmuls(2 * kb + jo, wq_coarse[kb - 1], jo)

        # ---- Q drains (DVE/ACT split, descale 2^-17) + phase B + KV drains
        qn = [acts.tile([P, NH, HD], F16, tag=f"qn{m}", name=f"qn{m}")
              for m in range(NM)]

        def qn_copy(m):
            nc.vector.tensor_scalar_mul(
                qn[m][:, 0:8, :],
                ps_a[m][0][:].rearrange("p (h d) -> p h d", d=HD), DS)
            nc.scalar.activation(
                qn[m][:, 8:16, :],
                ps_a[m][1][:].rearrange("p (h d) -> p h d", d=HD),
                mybir.ActivationFunctionType.Copy, scale=DS)

        ps_b = []
        kn = []
        v_sb = []

        def b_phase(m):
            pb = ptile([P, KVD], 6 + m % 2, f"psb{m}")
            ps_b.append(pb)
            ms = slice(m * P, (m + 1) * P)
            ni = 0
            for kb in range(8):
                for jo in range(2):
                    kp = 2 * kb + jo
                    for pi, (vx, vw) in enumerate(((0, 0), (1, 0), (0, 1))):
                        nc.tensor.matmul(
                            pb[:], xt_sb[:, vx, 2 * kp:2 * kp + 2, ms],
                            wkv_tiles[kb][:, vw, 2 * jo:2 * jo + 2, :],
                            start=(ni == 0), stop=(ni == 3 * KP - 1),
                            perf_mode=DR)
                        ni += 1

        def kv_drain(m):
            knt = acts.tile([P, NKV, HD], F16, tag=f"kn{m}", name=f"kn{m}")
            nc.vector.tensor_scalar_mul(
                knt[:], ps_b[m][:, 0:256].rearrange("p (h d) -> p h d", d=HD),
                DS)
            kn.append(knt)
            vt = acts.tile([P, 256], F16, tag=f"v{m}", name=f"v{m}")
            nc.scalar.activation(vt[:], ps_b[m][:, 256:512],
                                 mybir.ActivationFunctionType.Copy, scale=DSV)
            v_sb.append(vt)

        qn_copy(3)      # frees banks 6/7 for the b_phase rotation first
        qn_copy(0)
        b_phase(0)
        kv_drain(0)
        qn_copy(1)
        qn_copy(2)

        # ---- rope helpers (fp16 math; rstd via ln/exp on ACT) ----
        def stats(src, nh, m, tag):
            sq = rope_tmp.tile([P, nh, HD], F16, tag=f"sq{nh}", bufs=1,
                               name=f"sq_{tag}")
            nc.vector.tensor_mul(sq[:], src[:], src[:])
            ssq = rope_tmp.tile([P, nh], F32, tag=f"ssq{nh}", bufs=2,
                                name=f"ssq_{tag}")
            nc.vector.reduce_sum(ssq[:], sq[:], axis=mybir.AxisListType.X)
            lnv = rope_tmp.tile([P, nh], F32, tag=f"lnv{nh}", bufs=2,
                                name=f"lnv_{tag}")
            nc.scalar.activation(lnv[:], ssq[:],
                                 mybir.ActivationFunctionType.Ln,
                                 bias=eps_t[:], scale=1.0 / HD)
            rstd = rope_tmp.tile([P, nh], F16, tag=f"rstd{nh}", bufs=2,
                                 name=f"rstd_{tag}")
            nc.scalar.activation(rstd[:], lnv[:],
                                 mybir.ActivationFunctionType.Exp, scale=-0.5)
            return rstd

        def rope_apply(src, rstd, m, nh, cf, sf, tag):
            qn2 = rope_tmp.tile([P, nh, HD], F16, tag=f"qn2_{nh}", bufs=2,
                                name=f"qn2_{tag}")
            nc.vector.tensor_mul(qn2[:], src[:],
                                 rstd[:, :, None].to_broadcast((P, nh, HD)))
            ctab = tabs_sb[:, m, cf, :]
            stab = tabs_sb[:, m, sf, :]
            o1 = rope_tmp.tile([P, nh, HD], F16, tag=f"o1_{nh}", bufs=1,
                               name=f"o1_{tag}")
            nc.vector.tensor_mul(o1[:], qn2[:],
                                 ctab[:, None, :].to_broadcast((P, nh, HD)))
            o2 = rope_tmp.tile([P, nh, HD], F16, tag=f"o2_{nh}", bufs=1,
                               name=f"o2_{tag}")
            nc.vector.tensor_mul(
                o2[:, :, 0:H2], qn2[:, :, H2:HD],
                stab[:, None, 0:H2].to_broadcast((P, nh, H2)))
            nc.vector.tensor_mul(
                o2[:, :, H2:HD], qn2[:, :, 0:H2],
                stab[:, None, H2:HD].to_broadcast((P, nh, H2)))
            out = rope_tmp.tile([P, nh * HD], F16, tag=f"ro_{nh}", bufs=2,
                                name=f"ro_{tag}")
            nc.vector.tensor_add(out[:], o1[:].rearrange("p h d -> p (h d)"),
                                 o2[:].rearrange("p h d -> p (h d)"))
            return out

        krT = {}
        qrT = {}
        rope_out_k = {}
        rope_out_q = {}

        def rope_k(m):
            """DVE/ACT-only: rms-norm + rope for k of chunk m (needs kv m)."""
            rstd_k = stats(kn[m], NKV, m, f"k{m}")
            rope_out_k[m] = rope_apply(kn[m], rstd_k, m, NKV, 2, 3, f"k{m}")

        def rope_q(m):
            """Needs only qn (phase-A drains) + tabs — hoistable early."""
            rstd_q = stats(qn[m], NH, m, f"q{m}")
            rope_out_q[m] = rope_apply(qn[m], rstd_q, m, NH, 0, 1, f"q{m}")

        def rope_kq(m):
            rope_k(m)
            rope_q(m)

        def trans_kq(m):
            """PE transposes + ACT psum drains for chunk m (needs
            rope_kq(m)); XBAR DMA transposes were tried and regress — their
            transfers queue behind 1.5us bulk weight DMAs on the shared DMA
            engines right when the S matmuls need them."""
            kr = rope_out_k[m]
            qr = rope_out_q[m]
            kps = ptile([P, 2, P], 4, f"krT_ps{m}", F16)
            for j in range(2):
                nc.tensor.matmul(kps[:, j, :], kr[:, j * P:(j + 1) * P],
                                 ident[:], is_transpose=True)
            kt = acts.tile([P, 2, P], F16, tag=f"krT{m}", name=f"krT{m}")
            nc.scalar.copy(kt[:], kps[:])
            krT[m] = kt
            qps = ptile([P, 8, P], 5, f"qrT_ps{m}", F16)
            for j in range(8):
                nc.tensor.matmul(qps[:, j, :],
                                 qr[:, 2 * j * HD:(2 * j + 2) * HD],
                                 ident[:], is_transpose=True)
            qt = acts.tile([P, 8, P], F16, tag=f"qrT{m}", name=f"qrT{m}")
            nc.scalar.copy(qt[:], qps[:])
            qrT[m] = qt

        # A^T accumulator (= SA * A): partition (g%2)*64+d, col c=(g//2)*4+i,
        # token t. at16 fp16 + on-device fp8 hi/lo split for the emit GEMM.
        at16 = acts.tile([P, 8, T], F16, tag="at16")
        at8h = acts.tile([P, 8, T], F8, tag="at8h")
        at8l = acts.tile([P, 8, T], F8, tag="at8l")

        # ---- group-batched attention, split into a scores half and an AV
        # half so other PE work (phase B tail, transposes, emits) can sit
        # between them and hide the exp->gpsimd chain latency ----
        attn_state = {}

        def attn_scores(m):
            sums = attn_tmp.tile([P, 4, 512], F32, tag="sums", bufs=1,
                                 name=f"sums{m}")
            # pair layout: partition half (g%2)*64 of column gp holds group
            # g's reciprocal, matching o_ps[gp]'s layout so one [128,512]
            # multiply normalizes both groups of a pair at once
            rcp = attn_tmp.tile([P, 2, 512], F32, tag="rcp", bufs=1,
                                name=f"rcp{m}")
            ests = []
            attn_state[m] = (rcp, {}, sums, ests)
            for g in range(4):
                s_ps = ptile([P, 512], g % 2, f"s{m}_{g}")
                nc.tensor.matmul(s_ps[:], identw_sb[:],
                                 mask_sb[:].rearrange("p v f q -> p v (f q)"),
                                 start=True, stop=False, perf_mode=DR)
                base = (g % 2) * HD
                gp = g // 2
                nc.tensor.matmul(s_ps[:],
                                 krT[m][base:base + HD, gp, :],
                                 qrT[m][base:base + HD, 4 * gp:4 * gp + 4, :],
                                 start=False, stop=True)
                est = attn_tmp.tile([P, 4, P], F16, tag="est", bufs=4,
                                    name=f"est{m}_{g}")
                nc.scalar.activation(est[:], s_ps[:],
                                     mybir.ActivationFunctionType.Exp)
                nc.gpsimd.partition_all_reduce(sums[:, g, :], est[:],
                                               channels=P,
                                               reduce_op=ReduceOp.add)
                ests.append(est)

        def attn_avs(m):
            rcp, o_ps, sums, ests = attn_state[m]
            for g in range(4):
                base = (g % 2) * HD
                gp = g // 2
                if gp not in o_ps:
                    # chunk 2's AV lands on the psb banks (6,7), idle since
                    # kv3: dodges the WAR on norm(1)'s o_ps (2,3) read that
                    # otherwise stalls AV2 ~1.7us behind the DVE queue
                    ob = (6, 7) if m == 2 else (2, 3)
                    o_ps[gp] = ptile([P, 512], ob[gp], f"o{m}_{gp}")
                nc.tensor.matmul(o_ps[gp][base:base + HD, :],
                                 v_sb[m][:, g * HD:(g + 1) * HD], ests[g][:],
                                 start=True, stop=True)
                if g % 2 == 1:
                    # per-pair reciprocals into the pair layout; lane-locked
                    # halves keep the DVE queue moving
                    nc.vector.reciprocal(rcp[0:HD, gp, :],
                                         sums[0:HD, g - 1, :])
                    nc.vector.reciprocal(rcp[HD:P, gp, :],
                                         sums[HD:P, g, :])

        def attn_core(m):
            attn_scores(m)
            attn_avs(m)

        def attn_norm(m):
            rcp, o_ps, _, _ = attn_state[m]
            ms = slice(m * P, (m + 1) * P)
            for gp in range(2):
                # one multiply normalizes both groups of the pair: partition
                # halves of o_ps/rcp line up with at16's (g%2) layout
                nc.vector.tensor_mul(
                    at16[:, 4 * gp:4 * gp + 4, ms],
                    o_ps[gp][:].rearrange("p (i t) -> p i t", t=P),
                    rcp[:, gp, :].rearrange("p (i t) -> p i t", t=P))
            # fp8 hi/lo split for the emit GEMM: cast on DVE, residual on
            # GPSIMD — keeps the ACT queue pure exp (its latency releases the
            # S psum banks) and the softmax-critical Pool sums unobstructed
            nc.vector.tensor_copy(at8h[:, :, ms], at16[:, :, ms])
            nc.gpsimd.tensor_sub(at8l[:, 0:4, ms], at16[:, 0:4, ms],
                                 at8h[:, 0:4, ms])
            nc.vector.tensor_sub(at8l[:, 4:8, ms], at16[:, 4:8, ms],
                                 at8h[:, 4:8, ms])

        # ---- emit: Y^T = Wo^T @ A^T (3-pass fp8 DoubleRow); two mo's pair
        # up per psum bank so one [P,512] copy drains them and the WAR
        # pipeline is 4 mo's deep ----
        def emit_mb(half, mb, bank, split_tail=False, wo_tile=None):
            c0 = half * 256
            tsl = slice(c0, c0 + 256)
            if wo_tile is not None:
                wo_m = wo_tile
            else:
                # mb 0/1 tiles stay resident ("wo01") for the final half-1
                # emits so the tail has no wo DMAs in front of its yt writes
                tag, bufs = ("wo01", 1) if mb < 1 else ("wo", 3)
                wo_m = wstream.tile([P, 2, 4, 8, P], F8, tag=tag, bufs=bufs,
                                    name=f"wo_m{half}_{mb}")
                nc.sync.dma_start(wo_m[:], wo[:, :, mb * 4:(mb + 1) * 4, :, :])
            ys = ystage.tile([P, 4, 256], F16, tag="ys", name="ys")
            for pair in range(2):
                ps = ptile([P, 2, 256], bank[pair], f"ps_y{half}_{mb}_{pair}")
                for sub2 in range(2):
                    sub = pair * 2 + sub2
                    # at8h-only passes first, at8l pass last, all within ONE
                    # contiguous start..stop group (interleaving OPEN groups
                    # in a bank silently zeroes partial sums on hw)
                    ni = 0
                    for (va, vw) in ((0, 0), (0, 1), (1, 0)):
                        at_op = at8h if va == 0 else at8l
                        for u in range(4):
                            nc.tensor.matmul(
                                ps[:, sub2, :],
                                wo_m[:, vw, sub, 2 * u:2 * u + 2, :],
                                at_op[:, 2 * u:2 * u + 2, tsl],
                                start=(ni == 0), stop=(ni == 11),
                                perf_mode=DR)
                            ni += 1
                # alternate copy engines by (mb+pair) parity so consecutive
                # same-bank drains never queue behind each other
                on_dve = (mb + pair) % 2 == 0
                ys_dst = ys[:, 2 * pair:2 * pair + 2, :] \
                    .rearrange("p i t -> p (i t)")
                ps_src = ps[:].rearrange("p i t -> p (i t)")
                if on_dve:
                    nc.vector.tensor_scalar_mul(ys_dst, ps_src, DS)
                else:
                    nc.scalar.activation(ys_dst, ps_src,
                                         mybir.ActivationFunctionType.Copy,
                                         scale=DS)
                if split_tail:
                    nc.sync.dma_start(
                        yt_r[:, mb * 4 + 2 * pair:mb * 4 + 2 * pair + 2,
                             c0:c0 + 256],
                        ys[:, 2 * pair:2 * pair + 2, :])
            if not split_tail:
                nc.sync.dma_start(yt_r[:, mb * 4:(mb + 1) * 4, c0:c0 + 256],
                                  ys[:])
            return wo_m

        # ---- schedule: rope chains and transposes overlap the tail of
        # phase B; attention starts the moment B's last matmul retires;
        # each chunk's normalize+fp8-split follows its core so half-0 emits
        # (tokens 0..255 = chunks 0,1) can interleave with chunk 2/3
        # attention and fill the PE gaps of the latency-bound softmax chain
        # attention chunks pipeline INTO phase B: rope chains (DVE/ACT)
        # issue one b_phase ahead of their PE transposes, scores' exp/sums
        # latency hides behind the next b_phase's matmuls, and the emit
        # stream starts right after the last AV chain
        rope_kq(0)
        b_phase(1)
        kv_drain(1)
        rope_kq(1)
        trans_kq(0)
        b_phase(2)
        kv_drain(2)
        rope_kq(2)
        trans_kq(1)
        attn_scores(0)
        rope_q(3)        # q-rope of chunk 3 needs no phase-B data: its DVE
                         # chain runs under b3 instead of crowding the
                         # norm-critical window after it
        b_phase(3)       # fills the exp->gpsimd latency of chunk 0
        kv_drain(3)
        attn_avs(0)
        attn_scores(1)
        trans_kq(2)      # after scores(1): its ACT copies stay behind S1's
                         # bank-releasing exps
        attn_norm(0)     # DVE: frees o_ps banks 2/3 before rope-k(3) queues
        rope_k(3)
        attn_avs(1)
        attn_scores(2)
        trans_kq(3)
        attn_norm(1)
        attn_avs(2)
        attn_scores(3)
        attn_norm(2)
        wo_01 = emit_mb(0, 0, (6, 7))
        attn_avs(3)
        emit_mb(0, 1, (4, 5))
        attn_norm(3)
        # late mb stages cover both token halves from one wo load (wo would
        # otherwise be streamed twice: ~15us of DMA traffic saved); half-0
        # rotates (6,7)/(4,5) and half-1 (0,1)/(2,3) so four banks pipeline
        H1B = [(0, 1), (2, 3)]
        H0B = [(6, 7), (4, 5)]
        wo_t11 = None
        for mb in range(2, 8):
            wo_t = emit_mb(0, mb, H0B[mb % 2])
            if mb == 7:
                # prefetch the mb1 reload while the mb7 emits run so the
                # tail has no wo DMA in front of its yt writes
                wo_t11 = wstream.tile([P, 2, 4, 8, P], F8, tag="wo", bufs=3,
                                      name="wo_m1_reload")
                nc.sync.dma_start(wo_t11[:], wo[:, :, 4:8, :, :])
            emit_mb(1, mb, H1B[mb % 2], wo_tile=wo_t)
        emit_mb(1, 0, (0, 1), split_tail=True, wo_tile=wo_01)
        emit_mb(1, 1, (2, 3), split_tail=True, wo_tile=wo_t11)

    nc.finalize()
    return nc


def _split8(a, s):
    """fp8 hi/lo residual split at common scale s: a*s ~= hi + lo."""
    hi = (a * s).astype(E4)
    lo = (a * s - hi.astype(np.float32)).astype(E4)
    return hi, lo


def host_inputs(inputs, core):
    """Build the per-core DRAM input map from full problem inputs."""
    hs = np.asarray(inputs["hidden_states"], np.float32)
    am = np.asarray(inputs["attention_mask"], np.float32)
    cos = np.asarray(inputs["cos"], np.float32)
    sin = np.asarray(inputs["sin"], np.float32)
    Wqkv = np.asarray(inputs["Wqkv"], np.float32)
    Wo = np.asarray(inputs["Wo"], np.float32)
    qw = np.asarray(inputs["q_norm_w"], np.float32)
    kw = np.asarray(inputs["k_norm_w"], np.float32)

    LS = 256
    ls = slice(core * LS, (core + 1) * LS)
    X = hs[:, ls, :].reshape(T, HID)
    xt_f = np.ascontiguousarray(X.T)                      # [HID, T]
    xh, xl = _split8(xt_f, SX)
    # pack [HID, T] -> [P, 2, KO, T]
    xt8 = np.stack([xh.reshape(KO, P, T), xl.reshape(KO, P, T)], axis=0) \
        .transpose(2, 0, 1, 3)

    cos_c = cos[:, ls, :].reshape(T, HD)
    sin_c = sin[:, ls, :].reshape(T, HD)
    sq = float(HD) ** -0.25  # sqrt(1/sqrt(HD)) = sqrt(1/8)
    swap = np.concatenate([np.arange(32, 64), np.arange(0, 32)])
    sign = np.concatenate([-np.ones(32, np.float32), np.ones(32, np.float32)])

    tabs = np.empty((T, 4, HD), np.float32)
    tabs[:, 0, :] = cos_c * qw[None, :] * sq
    tabs[:, 1, :] = sin_c * qw[swap][None, :] * sign[None, :] * sq
    tabs[:, 2, :] = cos_c * kw[None, :] * sq
    tabs[:, 3, :] = sin_c * kw[swap][None, :] * sign[None, :] * sq

    # fp8 DoubleRow mask: v0 holds mask/16 (0 or -240), v1 zeros; the
    # stationary identw v0 is 16*I so the product restores -3840 (exp -> 0)
    maskT8 = np.where(am[0, 0, :P, :P].T < -1.0, -240.0, 0.0).astype(E4)
    mask8 = np.zeros((P, 2, 4, P), E4)
    mask8[:, 0, :, :] = np.broadcast_to(maskT8[:, None, :], (P, 4, P))
    identw = np.zeros((P, 2, P), E4)
    identw[:, 0, :] = (np.eye(P, dtype=np.float32) * 16.0).astype(E4)

    wq_f = np.ascontiguousarray(
        Wqkv[:, :QD].reshape(HID, NH, HD)[:, PERM, :].reshape(HID, QD))
    wqh, wql = _split8(wq_f, SW)
    wq8 = np.stack([wqh.reshape(KO, P, QD), wql.reshape(KO, P, QD)],
                   axis=0).transpose(2, 0, 1, 3)

    wkv_f = Wqkv[:, QD:]
    wkh, wkl = _split8(wkv_f, SW)
    wkv8 = np.stack([wkh.reshape(KO, P, KVD), wkl.reshape(KO, P, KVD)],
                    axis=0).transpose(2, 0, 1, 3)

    # wo[p=(par,d), mo, c, j] = Wo[h(c,par)*64+d, mo*128+j]
    woh = Wo.reshape(NH, HD, 32, P)
    wo_np = np.empty((P, 32, 8, P), np.float32)
    for par in range(2):
        for c in range(8):
            h = 8 * (c // 4) + 4 * par + (c % 4)
            wo_np[par * 64:(par + 1) * 64, :, c, :] = woh[h]
    woh8, wol8 = _split8(wo_np, SW)
    wo8 = np.stack([woh8, wol8], axis=1)                   # [P, 2, 32, 8, P]

    m = {
        "xt": np.ascontiguousarray(xt8),
        "tabs": np.ascontiguousarray(tabs.reshape(T, 4 * HD)).astype(np.float16),
        "wq": np.ascontiguousarray(wq8),
        "wkv": np.ascontiguousarray(wkv8),
        "wo": np.ascontiguousarray(wo8),
        "mask4": np.ascontiguousarray(mask8.reshape(P, 2 * 4 * P)),
        "identw": np.ascontiguousarray(identw.reshape(P, 2 * P)),
    }
    return m


def assemble_output(yts):
    """yts: list of 8 [4096, 512] fp16 arrays -> [2, 2048, 4096] f32."""
    out = np.empty((2, 2048, HID), np.float32)
    for c, yt_ in enumerate(yts):
        sl = yt_.astype(np.float32).T.reshape(2, 256, HID)
        out[:, c * 256:(c + 1) * 256, :] = sl
    return out


_NC_CACHE = {}


def _get_nc():
    if "nc" not in _NC_CACHE:
        _NC_CACHE["nc"] = build_nc()
    return _NC_CACHE["nc"]


def _run(inputs, trace=False):
    from concourse.bass_utils import run_bass_kernel_spmd
    nc = _get_nc()
    in_maps = [host_inputs(inputs, c) for c in range(8)]
    res = run_bass_kernel_spmd(nc, in_maps, core_ids=list(range(8)),
                               trace=trace)
    out = assemble_output([res.results[c]["yt"] for c in range(8)])
    return out, res


def kernel(**inputs):
    out, _ = _run(inputs, trace=False)
    if not np.isfinite(out).all():
        # transient first-execution flake seen once on device; retry
        out, _ = _run(inputs, trace=False)
    return out


def _timed_runs(inputs, n=20):
    """Amortized per-execution wall time (ns) of the compiled SPMD body with
    device-resident inputs. Used by test.py; not part of the grading path."""
    import time
    import jax
    from jax.sharding import Mesh, PartitionSpec, NamedSharding
    from jax.experimental.shard_map import shard_map
    import concourse.bass2jax as b2j
    import concourse.mybir as _mb

    nc = _get_nc()
    in_maps = [host_inputs(inputs, c) for c in range(8)]
    n_cores = 8
    b2j.install_neuronx_cc_hook()
    pname = nc.partition_id_tensor.name if nc.partition_id_tensor else None
    in_names, out_names, out_avals, zero_outs = [], [], [], []
    for alloc in nc.m.functions[0].allocations:
        if not isinstance(alloc, _mb.MemoryLocationSet):
            continue
        name = alloc.memorylocations[0].name
        if alloc.kind == "ExternalInput":
            if name != pname:
                in_names.append(name)
        elif alloc.kind == "ExternalOutput":
            out_names.append(name)
            shape = tuple(alloc.tensor_shape)
            dtype = _mb.dt.np(alloc.dtype)
            out_avals.append(jax.core.ShapedArray(shape, dtype))
            zero_outs.append(np.zeros(shape, dtype))
    n_params = len(in_names)
    all_in = list(in_names) + list(out_names)
    if pname is not None:
        all_in.append(pname)

    def _body(*args):
        operands = list(args)
        if pname is not None:
            operands.append(b2j.partition_id_tensor())
        return tuple(b2j._bass_exec_p.bind(
            *operands, out_avals=tuple(out_avals), in_names=tuple(all_in),
            out_names=tuple(out_names), lowering_input_output_aliases=(),
            sim_require_finite=True, sim_require_nnan=True, nc=nc))

    devices = jax.devices()[:n_cores]
    mesh = Mesh(np.asarray(devices), ("core",))
    specs = (PartitionSpec("core"),) * (n_params + len(out_names))
    fn = jax.jit(shard_map(_body, mesh=mesh, in_specs=specs,
                           out_specs=(PartitionSpec("core"),) * len(out_names),
                           check_rep=False), keep_unused=True)
    per_core = [[np.asarray(m[nm]) for nm in in_names] for m in in_maps]
    concat_in = [np.concatenate([per_core[c][i] for c in range(n_cores)])
                 for i in range(n_params)]
    concat_zero = [np.zeros((n_cores * z.shape[0], *z.shape[1:]), z.dtype)
                   for z in zero_outs]
    sh = NamedSharding(mesh, PartitionSpec("core"))
    dev_in = [jax.device_put(a, sh) for a in concat_in + concat_zero]
    out = fn(*dev_in)
    jax.block_until_ready(out)
    best = None
    for _ in range(3):
        t0 = time.time()
        for _ in range(n):
            out = fn(*dev_in)
        jax.block_until_ready(out)
        dt = (time.time() - t0) / n * 1e9
        best = dt if best is None else min(best, dt)
    return best
